# revision 5
# baseline (speedup 1.0000x reference)
"""Distributed GAT kernel for Trainium2 (8 NeuronCores), Bass/Tile. v2.

Architecture (per layer):
  - node tables [TROW, 256] f16 rows = [h(128 f16) | as(4 f32 as 8 f16) | pad]
    split into 4 quarter-tables; each quarter AllGather'd separately (Shared
    outputs) and triggered just before the edge pass that consumes it, so
    collectives overlap edge compute.
  - each core owns a dst shard; edges grouped by (src-quarter g, dst block j
    of 128 local dsts), chunked into 128-edge chunks (count = cross-core max).
  - per segment (24 chunks): dma_gather 512B rows by src; per chunk the
    TRANSPOSED one-hot dst mask (static!) is streamed from DRAM and the plain
    mask built by one DVE is_equal; ad via maskT matmul from SBUF ad table
    into one PSUM tile; batched e=lrelu(as+ad), ex=exp(e), weight msgs by ex;
    per chunk reduce matmul lhsT=mask rhs=[msgs|ex] accumulated per (g,j).
  - self-loop term computed densely from the local node table (initializes
    the accumulators), so self edges are not in the edge stream.
  - finalize: normalize by denom, +bias, ELU, transform with next W (f16).
  - layer 3: mean heads, +b3, ELU, dot lin_w, pool via Bpool matmul -> [64]
    partial per core; host sums partials (+lin_b).
"""
import numpy as np
import ml_dtypes
from contextlib import ExitStack

import concourse.bacc as bacc
import concourse.bass as bass
import concourse.tile as tile
from concourse import mybir, bass_utils
from concourse.library_config import mlp

F16 = mybir.dt.float16
F32 = mybir.dt.float32
F8 = mybir.dt.float8e4
I16 = mybir.dt.int16
NCORES = 8
P = 128
SEGC = 24          # chunks per gather segment
HEADS = 4
HID = 32
D1 = 128
NEG = 0.2
PAD_AS = -200.0
AGAP = 6
RB = 25            # real dst-blocks per table quarter
QROW = (RB + 1) * 128   # rows per quarter (+1 pad block) = 3328
TROW = 4 * QROW         # per-core table rows = 13312
GSZ = NCORES * QROW     # sub-table rows per quarter group = 26624
PADIDX = RB * 128       # pad row (core 0's pad block) within any sub-table


# ----------------------------------------------------------------------------
# host preprocessing
# ----------------------------------------------------------------------------
def preprocess(edge_index, N):
    """Build the core-independent schedule + per-core index/mask arrays.

    Chunks are 128 gathered edge-slots; a chunk may contain edges of several
    dst blocks ("parts"). Per part, transposed/plain one-hot masks select that
    part's edges (other slots 0), so per-(g,j) slot counts need no 128-ceil
    padding and segments need no dummy chunks.
    """
    SH = N // NCORES
    assert SH * NCORES == N
    NBLK = (SH + 127) // 128
    assert NBLK <= 4 * RB
    assert GSZ <= 32768

    src = edge_index[0].astype(np.int64)
    dst = edge_index[1].astype(np.int64)

    per_core = []
    cnt = np.zeros((NCORES, 4, NBLK), np.int64)
    for c in range(NCORES):
        m = (dst // SH) == c
        s_c, d_c = src[m], dst[m] - c * SH
        cs, i = s_c // SH, s_c % SH
        b, p = i // 128, i % 128
        g_c = b // RB
        row = cs * QROW + (b % RB) * 128 + p   # row within sub-table g
        order = np.lexsort((row, d_c, g_c))
        per_core.append((row[order], d_c[order], g_c[order]))
        np.add.at(cnt[c], (g_c[order], d_c[order] // 128), 1)

    maxcnt = cnt.max(axis=0)          # [4, NBLK] slots per (g, j) run

    # pack runs into 128-slot chunks (runs may split across chunks)
    chunk_parts = []   # per chunk: list of (g, j, lo, hi, rfirst, rlast)
    seg_bounds = []    # (c0, nch, p0, npar) per segment
    run_pos = {}       # (g, j) -> global slot start
    cbase = 0
    for g in range(4):
        pos = 0
        for j in range(NBLK):
            mc = int(maxcnt[g, j])
            if mc == 0:
                continue
            run_pos[(g, j)] = cbase * 128 + pos
            ch0, ch1 = pos // 128, (pos + mc - 1) // 128
            for ch in range(ch0, ch1 + 1):
                while cbase + ch >= len(chunk_parts):
                    chunk_parts.append([])
                lo = max(pos, ch * 128) - ch * 128
                hi = min(pos + mc, (ch + 1) * 128) - ch * 128
                chunk_parts[cbase + ch].append(
                    (g, j, lo, hi, ch == ch0, ch == ch1))
            pos += mc
        nch_pass = (pos + 127) // 128
        s = 0
        while s < nch_pass:
            nch = min(SEGC, nch_pass - s)
            seg_bounds.append((cbase + s, nch))
            s += nch
        cbase += nch_pass
    C = len(chunk_parts)

    # part program order + segment part offsets
    part_list = []     # (g, j, ci, lo, hi, rfirst, rlast)
    pmap = {}          # (g, j, ci) -> part index
    for ci, parts in enumerate(chunk_parts):
        for (g, j, lo, hi, fi, la) in parts:
            pmap[(g, j, ci)] = len(part_list)
            part_list.append((g, j, ci, lo, hi, fi, la))
    PARTS = len(part_list)
    segs = []
    for (c0, nch) in seg_bounds:
        p0 = pmap[chunk_parts[c0][0][:2] + (c0,)]
        npar = sum(len(chunk_parts[c]) for c in range(c0, c0 + nch))
        segs.append((c0, nch, p0, npar))
    SEGP = max(np_ for _, _, _, np_ in segs)

    idx_arrs, d8_arrs, mt_arrs = [], [], []
    for c in range(NCORES):
        s_row, d_l, g_c = per_core[c]
        idx = np.full((C, 128), PADIDX, np.int64)
        d8p = np.full((PARTS, 128), 255, np.int64)
        for g in range(4):
            mg = g_c == g
            sg, dg = s_row[mg], d_l[mg]
            blocks = dg // 128
            for j in np.unique(blocks):
                mb = blocks == j
                rows, dl = sg[mb], dg[mb]
                S0 = run_pos[(g, int(j))]
                n = len(rows)
                gs = S0 + np.arange(n)
                cis, sic = gs // 128, gs % 128
                idx[cis, sic] = rows
                ch0 = S0 // 128
                ch1 = (S0 + int(maxcnt[g, j]) - 1) // 128
                pis_by_ci = np.array(
                    [pmap[(g, int(j), ci)] for ci in range(ch0, ch1 + 1)])
                d8p[pis_by_ci[cis - ch0], sic] = dl - int(j) * 128
        # dma_gather layout: idx i -> partition i%16, col i//16; replicate x8
        flat = idx.reshape(-1)
        il = np.zeros((16, C * 8), np.int16)
        ar = np.arange(C * 128)
        il[ar % 16, ar // 16] = flat.astype(np.int16)
        idx_arrs.append(np.tile(il, (8, 1)))
        d8_arrs.append(np.ascontiguousarray(d8p.T).astype(np.float32))
        mt = (np.arange(128)[:, None, None] == d8p[None, :, :]
              ).astype(ml_dtypes.float8_e4m3)
        mt_arrs.append(np.ascontiguousarray(mt.reshape(128, PARTS * 128)))
    # per quarter q: first segment index by whose end all last-pass (g=3)
    # retires of quarter q's blocks have been emitted
    last_seg_of_run = {}
    for si, (c0, nch, p0, npar) in enumerate(segs):
        for ci in range(c0, c0 + nch):
            for (g, j, lo, hi, fi, la) in chunk_parts[ci]:
                if la:
                    last_seg_of_run[(g, j)] = si
    triggers = []
    for q in range(4):
        rb = min(RB, NBLK - q * RB)
        t = 0
        for j in range(q * RB, q * RB + rb):
            if (3, j) in last_seg_of_run:
                t = max(t, last_seg_of_run[(3, j)])
        triggers.append(t)
    sched = dict(SH=SH, NBLK=NBLK, C=C, PARTS=PARTS, SEGP=SEGP,
                 chunk_parts=chunk_parts, segs=segs, triggers=triggers)
    return sched, idx_arrs, d8_arrs, mt_arrs


# ----------------------------------------------------------------------------
# device program
# ----------------------------------------------------------------------------
def build_program(sched, repeat=1, variant=()):
    SH, NBLK, C = sched["SH"], sched["NBLK"], sched["C"]
    PARTS, SEGP = sched["PARTS"], sched["SEGP"]
    chunk_parts, segs = sched["chunk_parts"], sched["segs"]
    triggers = sched["triggers"]
    NROW = NBLK * 128

    nc = bacc.Bacc("TRN2", target_bir_lowering=False, debug=False,
                   num_devices=NCORES, num_swdge_queues=4)

    xT = nc.dram_tensor("xT", [P, NROW], F16, kind="ExternalInput")
    idx16 = nc.dram_tensor("idx16", [P, C * 8], I16, kind="ExternalInput")
    d8col = nc.dram_tensor("d8col", [P, PARTS], F32, kind="ExternalInput")
    maskt = nc.dram_tensor("maskt", [P, PARTS * 128], F8,
                           kind="ExternalInput")
    WT = [nc.dram_tensor(f"WT{l}", [P, P], F16, kind="ExternalInput")
          for l in range(3)]
    asrep = [nc.dram_tensor(f"asrep{l}", [P, P], F16, kind="ExternalInput")
             for l in range(3)]
    adrep = [nc.dram_tensor(f"adrep{l}", [P, P], F16, kind="ExternalInput")
             for l in range(3)]
    brep = [nc.dram_tensor(f"brep{l}", [P, P], F32, kind="ExternalInput")
            for l in range(2)]
    b3rep = nc.dram_tensor("b3rep", [P, HID], F32, kind="ExternalInput")
    lwrep = nc.dram_tensor("lwrep", [P, HID], F32, kind="ExternalInput")
    iotarep = nc.dram_tensor("iotarep", [P, P], F16, kind="ExternalInput")
    ident32 = nc.dram_tensor("ident32", [P, P], F32, kind="ExternalInput")
    padblk = nc.dram_tensor("padblk", [P, 256], F16, kind="ExternalInput")
    bpool = nc.dram_tensor("bpool", [P, NBLK * 64], F32, kind="ExternalInput")
    out64 = nc.dram_tensor("out64", [64, 1], F32, kind="ExternalOutput")

    with tile.TileContext(nc) as tc, ExitStack() as ctx:
        sb = ctx.enter_context(tc.tile_pool(name="sb", bufs=2))
        sbc = ctx.enter_context(tc.tile_pool(name="sbc", bufs=1))
        ps = ctx.enter_context(tc.tile_pool(name="ps", bufs=2, space="PSUM"))
        dr = ctx.enter_context(tc.tile_pool(name="dr", bufs=1, space="DRAM"))

        nc.gpsimd.load_library(mlp)

        iota_sb = sbc.tile([P, P], F16)
        nc.sync.dma_start(iota_sb[:], iotarep[:])
        id32_sb = sbc.tile([P, P], F32)
        nc.sync.dma_start(id32_sb[:], ident32[:])
        acc = sbc.tile([P, NBLK, 132], F32)
        ad_sb = sbc.tile([P, NBLK, 4], F16)
        hnode = sbc.tile([P, NBLK, P], F16)
        yv = acc[:, :, 0:128]

        tbl_in = dr.tile([TROW, 256], F16)
        hT = dr.tile([P, NROW], F16, name="hTd")

        wt_sb = [sbc.tile([P, P], F16, tag=f"wt{l}", name=f"wt{l}")
                 for l in range(3)]
        as_sb = [sbc.tile([P, P], F16, tag=f"asw{l}", name=f"asw{l}")
                 for l in range(3)]
        adw_sb = [sbc.tile([P, P], F16, tag=f"adw{l}", name=f"adw{l}")
                  for l in range(3)]
        b_sb = [sbc.tile([P, P], F32, tag=f"bb{l}", name=f"bb{l}")
                for l in range(2)]
        b3_sb = sbc.tile([P, HID], F32)
        lw_sb = sbc.tile([P, HID], F32)
        for l in range(3):
            nc.sync.dma_start(wt_sb[l][:], WT[l][:])
            nc.sync.dma_start(as_sb[l][:], asrep[l][:])
            nc.sync.dma_start(adw_sb[l][:], adrep[l][:])
        for l in range(2):
            nc.sync.dma_start(b_sb[l][:], brep[l][:])
        nc.sync.dma_start(b3_sb[:], b3rep[:])
        nc.sync.dma_start(lw_sb[:], lwrep[:])

        def w_transform_x(q, rb):
            pos = q * RB * 128
            end = pos + rb * 128
            while pos < end:
                w = min(512, end - pos)
                xs = sb.tile([P, 512], F16, tag="xs")
                nc.sync.dma_start(xs[:, :w], xT[:, pos:pos + w])
                hps = ps.tile([P, 512], F32, tag="big", space="PSUM")
                nc.tensor.matmul(hps[:, :w], lhsT=wt_sb[0][:],
                                 rhs=xs[:, :w], start=True, stop=True)
                hs = sb.tile([P, 512], F16, tag="hstage")
                nc.scalar.copy(hs[:, :w], hps[:, :w])
                nc.sync.dma_start(hT[:, pos:pos + w], hs[:, :w])
                pos += w

        def w_transform_y(l, q, rb):
            # fused: transpose yv (4 blocks) -> ytc chunk -> W matmul -> hT
            for b0 in range(0, rb, 4):
                s0 = q * RB + b0
                nb = min(4, rb - b0)
                ytc = sb.tile([P, 512], F16, tag="ytc")
                for t in range(nb):
                    tp = ps.tile([P, P], F32, tag="big", space="PSUM")
                    nc.tensor.transpose(out=tp[:], in_=yv[:, s0 + t, :],
                                        identity=id32_sb[:])
                    nc.scalar.copy(ytc[:, t * P:(t + 1) * P], tp[:])
                hps = ps.tile([P, 512], F32, tag="big", space="PSUM")
                nc.tensor.matmul(hps[:, :nb * P], lhsT=wt_sb[l][:],
                                 rhs=ytc[:, :nb * P], start=True, stop=True)
                hs = sb.tile([P, 512], F16, tag="hstage")
                nc.scalar.copy(hs[:, :nb * P], hps[:, :nb * P])
                nc.sync.dma_start(hT[:, s0 * P:s0 * P + nb * P],
                                  hs[:, :nb * P])

        def build_tables(l, q, rb):
            b0, r0 = q * RB, q * QROW
            nc.sync.dma_start_transpose(
                hnode[:, b0:b0 + rb, :],
                hT[:, b0 * 128:(b0 + rb) * 128])
            as_node = sb.tile([P, RB, 4], F32, tag="asred")
            ad_node = sb.tile([P, RB, 4], F32, tag="adred")
            QB = 13
            for rep, red in ((as_sb[l], as_node), (adw_sb[l], ad_node)):
                for q0 in range(0, rb, QB):
                    qn = min(QB, rb - q0)
                    tmp = sb.tile([P, QB, P], F16, tag="ashtmp", bufs=1)
                    nc.vector.tensor_tensor(
                        out=tmp[:, :qn, :], in0=hnode[:, b0 + q0:b0 + q0 + qn, :],
                        in1=rep[:].unsqueeze(1).to_broadcast([P, qn, P]),
                        op=mybir.AluOpType.mult)
                    nc.vector.tensor_reduce(
                        out=red[:, q0:q0 + qn, :],
                        in_=tmp[:, :qn, :].rearrange("p s (h c) -> p s h c", h=4),
                        axis=mybir.AxisListType.X, op=mybir.AluOpType.add)
            nc.vector.tensor_copy(ad_sb[:, b0:b0 + rb, :], ad_node[:, :rb, :])
            # quarter-table writes (h, as, pad blocks)
            nc.sync.dma_start(
                tbl_in[r0:r0 + rb * 128, 0:P].rearrange(
                    "(s p) f -> p s f", p=P),
                hnode[:, b0:b0 + rb, :])
            nc.sync.dma_start(
                tbl_in[:].bitcast(F32)[r0:r0 + rb * 128, 64:68].rearrange(
                    "(s p) f -> p s f", p=P),
                as_node[:, :rb, :])
            for t in range(rb, RB + 1):
                nc.sync.dma_start(
                    tbl_in[r0 + t * 128:r0 + (t + 1) * 128, :].rearrange(
                        "(s p) f -> p s f", p=P),
                    padblk[:].unsqueeze(1))
            # self-loop term: init acc with exp(lrelu(as+ad)) weighted h
            es = sb.tile([P, RB, 4], F32, tag="es")
            nc.vector.tensor_tensor(out=es[:, :rb, :], in0=as_node[:, :rb, :],
                                    in1=ad_node[:, :rb, :],
                                    op=mybir.AluOpType.add)
            nc.vector.scalar_tensor_tensor(
                out=es[:, :rb, :], in0=es[:, :rb, :], scalar=NEG,
                in1=es[:, :rb, :],
                op0=mybir.AluOpType.mult, op1=mybir.AluOpType.max)
            exs = sb.tile([P, RB, 4], F16, tag="exs")
            nc.scalar.activation(exs[:, :rb, :], es[:, :rb, :],
                                 mybir.ActivationFunctionType.Exp)
            nc.vector.tensor_copy(acc[:, b0:b0 + rb, 128:132], exs[:, :rb, :])
            nc.vector.tensor_tensor(
                out=yv[:, b0:b0 + rb, :].rearrange("p s (h c) -> p s h c", h=4),
                in0=hnode[:, b0:b0 + rb, :].rearrange(
                    "p s (h c) -> p s h c", h=4),
                in1=exs[:, :rb, :].unsqueeze(3).to_broadcast([P, rb, 4, 32]),
                op=mybir.AluOpType.mult)

        def collective(q, tq):
            if "noag" in variant:
                return
            nc.gpsimd.collective_compute(
                "AllGather", mybir.AluOpType.bypass,
                replica_groups=[list(range(NCORES))],
                ins=[tbl_in[q * QROW:(q + 1) * QROW, :].opt()],
                outs=[tq[q][:].opt()])

        def run_edges(tq, pro_cbs=()):
            # pro_cbs: list of (trigger_seg, ag_seg, fn, ag_fn); fn fires
            # after the trigger segment's scatters are emitted, ag_fn a few
            # segments later (keeps the collective off the Pool queue head
            # until its inputs are nearly ready)
            grp_ps = {}
            prev = None
            pending = sorted(pro_cbs, key=lambda t: t[0])
            scattered = -1
            for seg in list(segs) + [None]:
                if seg is not None:
                    c0, nch, p0, npar = seg
                    g = chunk_parts[c0][0][0]
                    msgs = sb.tile([P, SEGC, 256], F16, tag="msgs")
                    idx_sb = sb.tile([P, SEGC * 8], I16, tag="idxseg")
                    nc.sync.dma_start(idx_sb[:, :nch * 8],
                                      idx16[:, c0 * 8:(c0 + nch) * 8])
                    d8_sb = sb.tile([P, SEGP], F32, tag="d8seg")
                    nc.sync.dma_start(d8_sb[:, :npar],
                                      d8col[:, p0:p0 + npar])
                    mT8 = sb.tile([P, SEGP, P], F8, tag="mT8seg")
                    nc.sync.dma_start(
                        mT8[:, :npar, :],
                        maskt[:, p0 * 128:(p0 + npar) * 128])
                    mT = sb.tile([P, SEGP, P], F16, tag="mTseg")
                    nc.scalar.copy(mT[:, :npar, :], mT8[:, :npar, :])
                    if "nogather" not in variant:
                        # split across the 4 SWDGE queues: 4 rings per SDMA
                        # engine keep more HBM reads in flight (~2x faster
                        # than one ring for 512B random rows)
                        a = 0
                        qn = 0
                        while a < nch:
                            b = min(a + 6, nch)
                            nidx = (b - a) * 128
                            nc.gpsimd.dma_gather(
                                msgs[:, a:b, :], tq[g][:],
                                idx_sb[:, a * 8:b * 8],
                                nidx, nidx, 256, single_packet=False,
                                queue_num=qn)
                            qn = (qn + 1) % 4
                            a = b
                    maskS = sb.tile([P, SEGP, P], F16, tag="maskS", bufs=3)
                    if "nomask" not in variant:
                        for pp in range(npar):
                            nc.vector.tensor_scalar(
                                out=maskS[:, pp, :], in0=iota_sb[:],
                                scalar1=d8_sb[:, pp:pp + 1], scalar2=None,
                                op0=mybir.AluOpType.is_equal)
                    ad_ps = ps.tile([P, SEGC, 4], F32, tag="adps",
                                    space="PSUM")
                    if "nope" not in variant:
                        pp = 0
                        for k in range(nch):
                            parts = chunk_parts[c0 + k]
                            for t, (g2, j, lo, hi, fi, la) in enumerate(parts):
                                nc.tensor.matmul(
                                    ad_ps[:, k, :], lhsT=mT[:, pp, :],
                                    rhs=ad_sb[:, j, :], start=(t == 0),
                                    stop=(t == len(parts) - 1))
                                pp += 1
                    wmsg = sb.tile([P, SEGC, 132], F16, tag="wmsg")
                    e1 = sb.tile([P, SEGC, 4], F32, tag="e1")
                    nc.vector.tensor_tensor(
                        out=e1[:, :nch, :],
                        in0=msgs[:].bitcast(F32)[:, :nch, 64:68],
                        in1=ad_ps[:, :nch, :], op=mybir.AluOpType.add)
                    nc.vector.scalar_tensor_tensor(
                        out=e1[:, :nch, :], in0=e1[:, :nch, :], scalar=NEG,
                        in1=e1[:, :nch, :],
                        op0=mybir.AluOpType.mult, op1=mybir.AluOpType.max)
                    nc.scalar.activation(wmsg[:, :nch, 128:132],
                                         e1[:, :nch, :],
                                         mybir.ActivationFunctionType.Exp)
                    nc.vector.tensor_tensor(
                        out=wmsg[:, :nch, 0:128].rearrange(
                            "p s (h c) -> p s h c", h=4),
                        in0=msgs[:, :nch, 0:128].rearrange(
                            "p s (h c) -> p s h c", h=4),
                        in1=wmsg[:, :nch, 128:132].unsqueeze(3).to_broadcast(
                            [P, nch, 4, 32]),
                        op=mybir.AluOpType.mult)
                    cur = (c0, nch, maskS, wmsg)
                else:
                    cur = None
                # previous segment's scatter matmuls (PE overlaps the
                # exp/weight chain above with these accumulations)
                if prev is not None and "nope" not in variant:
                    pc0, pnch, pmask, pwmsg = prev
                    pp = 0
                    for k in range(pnch):
                        for (g2, j, lo, hi, fi, la) in chunk_parts[pc0 + k]:
                            key = (g2, j)
                            if fi:
                                grp_ps[key] = ps.tile([P, 132], F32,
                                                      tag="grp", name="grp",
                                                      space="PSUM")
                            gp = grp_ps[key]
                            nc.tensor.matmul(gp[:], lhsT=pmask[:, pp, :],
                                             rhs=pwmsg[:, k, :],
                                             start=fi, stop=la)
                            if la:
                                nc.vector.tensor_tensor(
                                    out=acc[:, j, :], in0=acc[:, j, :],
                                    in1=gp[:], op=mybir.AluOpType.add)
                            pp += 1
                    scattered += 1
                for (ts, tag_, fn, ag_fn) in pending:
                    if fn is not None and scattered >= ts:
                        fn()
                    if ag_fn is not None and (scattered >= tag_
                                              or cur is None):
                        ag_fn()
                pending = [(ts, tag_, None if (fn is None or scattered >= ts)
                            else fn,
                            None if (ag_fn is None or scattered >= tag_
                                     or cur is None) else ag_fn)
                           for (ts, tag_, fn, ag_fn) in pending]
                pending = [t for t in pending
                           if t[2] is not None or t[3] is not None]
                prev = cur

        def elu_inplace(full_ap, nblk, width):
            EB = 4
            for q0 in range(0, nblk, EB):
                qn = min(EB, nblk - q0)
                ap = full_ap[:, q0:q0 + qn, :]
                shape = [P, EB, width]
                a = sb.tile(shape, F32, tag="elua", bufs=1)
                nc.scalar.activation(a[:, :qn, :], ap,
                                     mybir.ActivationFunctionType.Relu)
                bmin = sb.tile(shape, F32, tag="elub", bufs=1)
                nc.vector.tensor_scalar(out=bmin[:, :qn, :], in0=ap,
                                        scalar1=0.0, scalar2=None,
                                        op0=mybir.AluOpType.min)
                cc = sb.tile(shape, F32, tag="eluc", bufs=1)
                nc.scalar.activation(cc[:, :qn, :], bmin[:, :qn, :],
                                     mybir.ActivationFunctionType.Exp)
                nc.vector.scalar_tensor_tensor(
                    out=ap, in0=a[:, :qn, :], scalar=-1.0, in1=cc[:, :qn, :],
                    op0=mybir.AluOpType.add, op1=mybir.AluOpType.add)

        def finalize(l, q, rb):
            b0 = q * RB
            rec = sb.tile([P, RB, 4], F32, tag="rec")
            nc.vector.reciprocal(out=rec[:, :rb, :],
                                 in_=acc[:, b0:b0 + rb, 128:132])
            nc.vector.tensor_tensor(
                out=yv[:, b0:b0 + rb, :].rearrange("p s (h c) -> p s h c", h=4),
                in0=acc[:, b0:b0 + rb, 0:128].rearrange(
                    "p s (h c) -> p s h c", h=4),
                in1=rec[:, :rb, :].unsqueeze(3).to_broadcast([P, rb, 4, 32]),
                op=mybir.AluOpType.mult)
            if l < 2:
                nc.vector.tensor_tensor(
                    out=yv[:, b0:b0 + rb, :], in0=yv[:, b0:b0 + rb, :],
                    in1=b_sb[l][:].unsqueeze(1).to_broadcast([P, rb, P]),
                    op=mybir.AluOpType.add)
                elu_inplace(yv[:, b0:b0 + rb, :], rb, P)

        def make_prologue(l, q, tq, first):
            rb = min(RB, NBLK - q * RB)

            def fn():
                if not first:
                    finalize(2 if l == 0 else l - 1, q, rb)
                if l == 0:
                    w_transform_x(q, rb)
                else:
                    w_transform_y(l, q, rb)
                build_tables(l, q, rb)

            def ag_fn():
                collective(q, tq)
            return fn, ag_fn

        nseg_all = len(segs)
        phases = [(r, l) for r in range(repeat) for l in range(3)]
        tqs = []
        for (r, l) in phases:
            tqs.append([dr.tile([GSZ, 256], F16, addr_space="Shared",
                                name=f"tq{r}_{l}_{q}", tag=f"tq{r}_{l}_{q}")
                        for q in range(4)])
        # phase 0's prologue emitted inline; phase i+1's inside run_edges(i)
        for q in range(4):
            fn, ag_fn = make_prologue(0, q, tqs[0], True)
            fn(); ag_fn()
        for i, (r, l) in enumerate(phases):
            cbs = []
            if i + 1 < len(phases):
                nl = phases[i + 1][1]
                for q in range(4):
                    fn, ag_fn = make_prologue(nl, q, tqs[i + 1], False)
                    cbs.append((triggers[q],
                                min(triggers[q] + AGAP, nseg_all - 1), fn,
                                ag_fn))
            run_edges(tqs[i], cbs)
        finalize(2, 0, RB); finalize(2, 1, RB); finalize(2, 2, RB)
        finalize(2, 3, NBLK - 3 * RB)

        h3 = sb.tile([P, NBLK, HID], F32, tag="h3", bufs=1)
        nc.vector.tensor_reduce(
            out=h3[:], in_=yv[:].rearrange("p s (h c) -> p s c h", h=4),
            axis=mybir.AxisListType.X, op=mybir.AluOpType.add)
        nc.vector.tensor_scalar(out=h3[:], in0=h3[:], scalar1=0.25,
                                scalar2=None, op0=mybir.AluOpType.mult)
        nc.vector.tensor_tensor(
            out=h3[:], in0=h3[:],
            in1=b3_sb[:].unsqueeze(1).to_broadcast([P, NBLK, HID]),
            op=mybir.AluOpType.add)
        elu_inplace(h3, NBLK, HID)
        pv = sb.tile([P, NBLK], F32, tag="pv", bufs=1)
        for q0 in range(0, NBLK, 16):
            qn = min(16, NBLK - q0)
            tmp3 = sb.tile([P, 16, HID], F32, tag="tmp3", bufs=1)
            nc.vector.tensor_tensor(
                out=tmp3[:, :qn, :], in0=h3[:, q0:q0 + qn, :],
                in1=lw_sb[:].unsqueeze(1).to_broadcast([P, qn, HID]),
                op=mybir.AluOpType.mult)
            nc.vector.tensor_reduce(out=pv[:, q0:q0 + qn], in_=tmp3[:, :qn, :],
                                    axis=mybir.AxisListType.X,
                                    op=mybir.AluOpType.add)
        pool_ps = ps.tile([64, 1], F32, tag="big", space="PSUM")
        for s in range(NBLK):
            bps = sb.tile([P, 64], F32, tag="bps")
            nc.sync.dma_start(bps[:], bpool[:, s * 64:(s + 1) * 64])
            nc.tensor.matmul(pool_ps[:], lhsT=bps[:], rhs=pv[:, s:s + 1],
                             start=(s == 0), stop=(s == NBLK - 1))
        pool_sb = sb.tile([64, 1], F32, tag="poolsb", bufs=1)
        nc.scalar.copy(pool_sb[:], pool_ps[:])
        nc.sync.dma_start(out64[:], pool_sb[:])

    nc.compile()
    return nc


# ----------------------------------------------------------------------------
# host-side input construction
# ----------------------------------------------------------------------------
def make_inputs(sched, idx_arrs, d8_arrs, mt_arrs, inputs, batch_counts=None):
    """Per-core in_maps from the raw problem inputs dict."""
    SH, NBLK = sched["SH"], sched["NBLK"]
    NROW = NBLK * 128
    x = np.asarray(inputs["x"], np.float32)
    N = x.shape[0]
    batch = np.asarray(inputs["batch"], np.int64)
    NGr = 64 if batch_counts is None else len(batch_counts)
    counts = np.bincount(batch, minlength=NGr).astype(np.float32)
    counts[counts == 0] = 1.0

    def rep(v, dt=np.float32):
        v = np.asarray(v, np.float32).reshape(1, -1)
        return np.tile(v, (P, 1)).astype(dt)

    Ws = [np.asarray(inputs[k], np.float32).T.astype(np.float16).copy()
          for k in ("W1", "W2", "W3")]
    asr = [rep(np.asarray(inputs[k], np.float32).reshape(-1), np.float16)
           for k in ("a1s", "a2s", "a3s")]
    adr = [rep(np.asarray(inputs[k], np.float32).reshape(-1), np.float16)
           for k in ("a1d", "a2d", "a3d")]
    br = [rep(inputs["b1"]), rep(inputs["b2"])]
    b3r = rep(inputs["b3"])
    lwr = rep(np.asarray(inputs["lin_w"], np.float32).reshape(-1))
    iot = np.tile(np.arange(P, dtype=np.float32), (P, 1)).astype(np.float16)
    idf32 = np.eye(P, dtype=np.float32)
    pad = np.zeros((P, 256), np.float16)
    pad.view(np.float32)[:, 64:68] = PAD_AS

    in_maps = []
    for c in range(NCORES):
        xs = np.zeros((NROW, P), np.float32)
        xs[0:SH] = x[c * SH:(c + 1) * SH]
        bp = np.zeros((NROW, 64), np.float32)
        b_loc = batch[c * SH:(c + 1) * SH]
        bp[np.arange(SH), b_loc] = 1.0 / counts[b_loc]
        m = {"xT": np.ascontiguousarray(xs.T).astype(np.float16),
             "idx16": idx_arrs[c], "d8col": d8_arrs[c], "maskt": mt_arrs[c],
             "b3rep": b3r, "lwrep": lwr, "iotarep": iot,
             "ident32": idf32, "padblk": pad,
             "bpool": np.ascontiguousarray(
                 bp.reshape(NBLK, P, 64).transpose(1, 0, 2).reshape(
                     P, NBLK * 64))}
        for l in range(3):
            m[f"WT{l}"] = Ws[l]
            m[f"asrep{l}"] = asr[l]
            m[f"adrep{l}"] = adr[l]
        for l in range(2):
            m[f"brep{l}"] = br[l]
        in_maps.append(m)
    return in_maps


# ----------------------------------------------------------------------------
# SPMD runner (modeled on bass2jax.run_bass_via_pjrt, with reusable executable)
# ----------------------------------------------------------------------------
def make_runner(nc, in_maps):
    import jax
    import jax.numpy as jnp
    from jax.sharding import Mesh, PartitionSpec
    from jax.experimental.shard_map import shard_map
    from concourse import bass2jax, mybir as mb

    bass2jax.install_neuronx_cc_hook()
    n_cores = len(in_maps)
    part_name = nc.partition_id_tensor.name if nc.partition_id_tensor else None
    in_names, out_names, out_avals, zero_outs = [], [], [], []
    for alloc in nc.m.functions[0].allocations:
        if not isinstance(alloc, mb.MemoryLocationSet):
            continue
        name = alloc.memorylocations[0].name
        if alloc.kind == "ExternalInput":
            if name != part_name:
                in_names.append(name)
        elif alloc.kind == "ExternalOutput":
            out_names.append(name)
            shape = tuple(alloc.tensor_shape)
            dtype = mb.dt.np(alloc.dtype)
            out_avals.append(jax.core.ShapedArray(shape, dtype))
            zero_outs.append(np.zeros(shape, dtype))
    n_params = len(in_names)
    all_names = in_names + out_names
    if part_name is not None:
        all_names = all_names + [part_name]

    def _body(*args):
        operands = list(args)
        if part_name is not None:
            operands.append(bass2jax.partition_id_tensor())
        outs = bass2jax._bass_exec_p.bind(
            *operands, out_avals=tuple(out_avals), in_names=tuple(all_names),
            out_names=tuple(out_names), lowering_input_output_aliases=(),
            sim_require_finite=False, sim_require_nnan=False, nc=nc)
        return tuple(outs)

    devices = jax.devices()[:n_cores]
    mesh = Mesh(np.asarray(devices), ("core",))
    in_specs = (PartitionSpec("core"),) * (n_params + len(out_names))
    out_specs = (PartitionSpec("core"),) * len(out_names)
    fn = jax.jit(shard_map(_body, mesh=mesh, in_specs=in_specs,
                           out_specs=out_specs, check_rep=False))
    concat_in = [np.concatenate([np.asarray(in_maps[c][nm])
                                 for c in range(n_cores)], axis=0)
                 for nm in in_names]
    concat_zeros = [np.zeros((n_cores * z.shape[0], *z.shape[1:]), z.dtype)
                    for z in zero_outs]
    dev_in = [jax.device_put(
        a, jax.sharding.NamedSharding(mesh, PartitionSpec("core")))
        for a in concat_in + concat_zeros]

    def run():
        outs = fn(*dev_in)
        outs = [np.asarray(o) for o in outs]
        return [
            {nm: outs[i].reshape(n_cores, *out_avals[i].shape)[c]
             for i, nm in enumerate(out_names)}
            for c in range(n_cores)]
    return run


def kernel(**inputs):
    """Full-input distributed GAT kernel; returns pooled [64] float32."""
    inputs = {k: np.asarray(v) for k, v in inputs.items()}
    N = inputs["x"].shape[0]
    sched, idx_arrs, d8_arrs, mt_arrs = preprocess(inputs["edge_index"], N)
    nc = build_program(sched)
    in_maps = make_inputs(sched, idx_arrs, d8_arrs, mt_arrs, inputs)
    run = make_runner(nc, in_maps)
    kernel.last_runner = run          # exposed for test.py timing
    kernel.last_inputs = inputs
    results = run()
    partial = sum(r["out64"][:, 0] for r in results)
    out = (partial + np.float32(inputs["lin_b"].reshape(-1)[0]))[:64]
    return out.astype(np.float32)



# revision 6
# speedup vs baseline: 1.0344x; 1.0344x over previous
"""Distributed GAT kernel for Trainium2 (8 NeuronCores), Bass/Tile. v2.

Architecture (per layer):
  - node tables [TROW, 256] f16 rows = [h(128 f16) | as(4 f32 as 8 f16) | pad]
    split into 4 quarter-tables; each quarter AllGather'd separately (Shared
    outputs) and triggered just before the edge pass that consumes it, so
    collectives overlap edge compute.
  - each core owns a dst shard; edges grouped by (src-quarter g, dst block j
    of 128 local dsts), chunked into 128-edge chunks (count = cross-core max).
  - per segment (24 chunks): dma_gather 512B rows by src; per chunk the
    TRANSPOSED one-hot dst mask (static!) is streamed from DRAM and the plain
    mask built by one DVE is_equal; ad via maskT matmul from SBUF ad table
    into one PSUM tile; batched e=lrelu(as+ad), ex=exp(e), weight msgs by ex;
    per chunk reduce matmul lhsT=mask rhs=[msgs|ex] accumulated per (g,j).
  - self-loop term computed densely from the local node table (initializes
    the accumulators), so self edges are not in the edge stream.
  - finalize: normalize by denom, +bias, ELU, transform with next W (f16).
  - layer 3: mean heads, +b3, ELU, dot lin_w, pool via Bpool matmul -> [64]
    partial per core; host sums partials (+lin_b).
"""
import numpy as np
import ml_dtypes
from contextlib import ExitStack

import concourse.bacc as bacc
import concourse.bass as bass
import concourse.tile as tile
from concourse import mybir, bass_utils
from concourse.library_config import mlp

F16 = mybir.dt.float16
F32 = mybir.dt.float32
F8 = mybir.dt.float8e4
I16 = mybir.dt.int16
NCORES = 8
P = 128
SEGC = 24          # chunks per gather segment
HEADS = 4
HID = 32
D1 = 128
NEG = 0.2
PAD_AS = -200.0
AGAP = 4
RB = 25            # real dst-blocks per table quarter
QROW = (RB + 1) * 128   # rows per quarter (+1 pad block) = 3328
TROW = 4 * QROW         # per-core table rows = 13312
GSZ = NCORES * QROW     # sub-table rows per quarter group = 26624
PADIDX = RB * 128       # pad row (core 0's pad block) within any sub-table


# ----------------------------------------------------------------------------
# host preprocessing
# ----------------------------------------------------------------------------
def preprocess(edge_index, N):
    """Build the core-independent schedule + per-core index/mask arrays.

    Chunks are 128 gathered edge-slots; a chunk may contain edges of several
    dst blocks ("parts"). Per part, transposed/plain one-hot masks select that
    part's edges (other slots 0), so per-(g,j) slot counts need no 128-ceil
    padding and segments need no dummy chunks.
    """
    SH = N // NCORES
    assert SH * NCORES == N
    NBLK = (SH + 127) // 128
    assert NBLK <= 4 * RB
    assert GSZ <= 32768

    src = edge_index[0].astype(np.int64)
    dst = edge_index[1].astype(np.int64)

    per_core = []
    cnt = np.zeros((NCORES, 4, NBLK), np.int64)
    for c in range(NCORES):
        m = (dst // SH) == c
        s_c, d_c = src[m], dst[m] - c * SH
        cs, i = s_c // SH, s_c % SH
        b, p = i // 128, i % 128
        g_c = b // RB
        row = cs * QROW + (b % RB) * 128 + p   # row within sub-table g
        order = np.lexsort((row, d_c, g_c))
        per_core.append((row[order], d_c[order], g_c[order]))
        np.add.at(cnt[c], (g_c[order], d_c[order] // 128), 1)

    maxcnt = cnt.max(axis=0)          # [4, NBLK] slots per (g, j) run

    # pack runs into 128-slot chunks (runs may split across chunks)
    chunk_parts = []   # per chunk: list of (g, j, lo, hi, rfirst, rlast)
    seg_bounds = []    # (c0, nch, p0, npar) per segment
    run_pos = {}       # (g, j) -> global slot start
    cbase = 0
    for g in range(4):
        pos = 0
        for j in range(NBLK):
            mc = int(maxcnt[g, j])
            if mc == 0:
                continue
            run_pos[(g, j)] = cbase * 128 + pos
            ch0, ch1 = pos // 128, (pos + mc - 1) // 128
            for ch in range(ch0, ch1 + 1):
                while cbase + ch >= len(chunk_parts):
                    chunk_parts.append([])
                lo = max(pos, ch * 128) - ch * 128
                hi = min(pos + mc, (ch + 1) * 128) - ch * 128
                chunk_parts[cbase + ch].append(
                    (g, j, lo, hi, ch == ch0, ch == ch1))
            pos += mc
        nch_pass = (pos + 127) // 128
        s = 0
        while s < nch_pass:
            nch = min(SEGC, nch_pass - s)
            seg_bounds.append((cbase + s, nch))
            s += nch
        cbase += nch_pass
    C = len(chunk_parts)

    # part program order + segment part offsets
    part_list = []     # (g, j, ci, lo, hi, rfirst, rlast)
    pmap = {}          # (g, j, ci) -> part index
    for ci, parts in enumerate(chunk_parts):
        for (g, j, lo, hi, fi, la) in parts:
            pmap[(g, j, ci)] = len(part_list)
            part_list.append((g, j, ci, lo, hi, fi, la))
    PARTS = len(part_list)
    segs = []
    for (c0, nch) in seg_bounds:
        p0 = pmap[chunk_parts[c0][0][:2] + (c0,)]
        npar = sum(len(chunk_parts[c]) for c in range(c0, c0 + nch))
        segs.append((c0, nch, p0, npar))
    SEGP = max(np_ for _, _, _, np_ in segs)

    idx_arrs, d8_arrs, mt_arrs = [], [], []
    for c in range(NCORES):
        s_row, d_l, g_c = per_core[c]
        idx = np.full((C, 128), PADIDX, np.int64)
        d8p = np.full((PARTS, 128), 255, np.int64)
        for g in range(4):
            mg = g_c == g
            sg, dg = s_row[mg], d_l[mg]
            blocks = dg // 128
            for j in np.unique(blocks):
                mb = blocks == j
                rows, dl = sg[mb], dg[mb]
                S0 = run_pos[(g, int(j))]
                n = len(rows)
                gs = S0 + np.arange(n)
                cis, sic = gs // 128, gs % 128
                idx[cis, sic] = rows
                ch0 = S0 // 128
                ch1 = (S0 + int(maxcnt[g, j]) - 1) // 128
                pis_by_ci = np.array(
                    [pmap[(g, int(j), ci)] for ci in range(ch0, ch1 + 1)])
                d8p[pis_by_ci[cis - ch0], sic] = dl - int(j) * 128
        # dma_gather layout: idx i -> partition i%16, col i//16; replicate x8
        flat = idx.reshape(-1)
        il = np.zeros((16, C * 8), np.int16)
        ar = np.arange(C * 128)
        il[ar % 16, ar // 16] = flat.astype(np.int16)
        idx_arrs.append(np.tile(il, (8, 1)))
        d8_arrs.append(np.ascontiguousarray(d8p.T).astype(np.float32))
        mt = (np.arange(128)[:, None, None] == d8p[None, :, :]
              ).astype(ml_dtypes.float8_e4m3)
        mt_arrs.append(np.ascontiguousarray(mt.reshape(128, PARTS * 128)))
    # per quarter q: first segment index by whose end all last-pass (g=3)
    # retires of quarter q's blocks have been emitted
    last_seg_of_run = {}
    for si, (c0, nch, p0, npar) in enumerate(segs):
        for ci in range(c0, c0 + nch):
            for (g, j, lo, hi, fi, la) in chunk_parts[ci]:
                if la:
                    last_seg_of_run[(g, j)] = si
    triggers = []
    for q in range(4):
        rb = min(RB, NBLK - q * RB)
        t = 0
        for j in range(q * RB, q * RB + rb):
            if (3, j) in last_seg_of_run:
                t = max(t, last_seg_of_run[(3, j)])
        triggers.append(t)
    sched = dict(SH=SH, NBLK=NBLK, C=C, PARTS=PARTS, SEGP=SEGP,
                 chunk_parts=chunk_parts, segs=segs, triggers=triggers)
    return sched, idx_arrs, d8_arrs, mt_arrs


# ----------------------------------------------------------------------------
# device program
# ----------------------------------------------------------------------------
def build_program(sched, repeat=1, variant=()):
    SH, NBLK, C = sched["SH"], sched["NBLK"], sched["C"]
    PARTS, SEGP = sched["PARTS"], sched["SEGP"]
    chunk_parts, segs = sched["chunk_parts"], sched["segs"]
    triggers = sched["triggers"]
    NROW = NBLK * 128

    nc = bacc.Bacc("TRN2", target_bir_lowering=False, debug=False,
                   num_devices=NCORES, num_swdge_queues=4)

    xT = nc.dram_tensor("xT", [P, NROW], F16, kind="ExternalInput")
    idx16 = nc.dram_tensor("idx16", [P, C * 8], I16, kind="ExternalInput")
    d8col = nc.dram_tensor("d8col", [P, PARTS], F32, kind="ExternalInput")
    maskt = nc.dram_tensor("maskt", [P, PARTS * 128], F8,
                           kind="ExternalInput")
    WT = [nc.dram_tensor(f"WT{l}", [P, P], F16, kind="ExternalInput")
          for l in range(3)]
    asrep = [nc.dram_tensor(f"asrep{l}", [P, P], F16, kind="ExternalInput")
             for l in range(3)]
    adrep = [nc.dram_tensor(f"adrep{l}", [P, P], F16, kind="ExternalInput")
             for l in range(3)]
    brep = [nc.dram_tensor(f"brep{l}", [P, P], F32, kind="ExternalInput")
            for l in range(2)]
    b3rep = nc.dram_tensor("b3rep", [P, HID], F32, kind="ExternalInput")
    lwrep = nc.dram_tensor("lwrep", [P, HID], F32, kind="ExternalInput")
    iotarep = nc.dram_tensor("iotarep", [P, P], F16, kind="ExternalInput")
    ident32 = nc.dram_tensor("ident32", [P, P], F32, kind="ExternalInput")
    padblk = nc.dram_tensor("padblk", [P, 256], F16, kind="ExternalInput")
    bpool = nc.dram_tensor("bpool", [P, NBLK * 64], F32, kind="ExternalInput")
    out64 = nc.dram_tensor("out64", [64, 1], F32, kind="ExternalOutput")

    with tile.TileContext(nc) as tc, ExitStack() as ctx:
        sb = ctx.enter_context(tc.tile_pool(name="sb", bufs=2))
        sbc = ctx.enter_context(tc.tile_pool(name="sbc", bufs=1))
        ps = ctx.enter_context(tc.tile_pool(name="ps", bufs=2, space="PSUM"))
        dr = ctx.enter_context(tc.tile_pool(name="dr", bufs=1, space="DRAM"))

        nc.gpsimd.load_library(mlp)

        iota_sb = sbc.tile([P, P], F16)
        nc.sync.dma_start(iota_sb[:], iotarep[:])
        id32_sb = sbc.tile([P, P], F32)
        nc.sync.dma_start(id32_sb[:], ident32[:])
        acc = sbc.tile([P, NBLK, 132], F32)
        ad_sb = sbc.tile([P, NBLK, 4], F16)
        hnode = sbc.tile([P, NBLK, P], F16)
        yv = acc[:, :, 0:128]

        tbl_in = dr.tile([TROW, 256], F16)
        hT = dr.tile([P, NROW], F16, name="hTd")

        wt_sb = [sbc.tile([P, P], F16, tag=f"wt{l}", name=f"wt{l}")
                 for l in range(3)]
        as_sb = [sbc.tile([P, P], F16, tag=f"asw{l}", name=f"asw{l}")
                 for l in range(3)]
        adw_sb = [sbc.tile([P, P], F16, tag=f"adw{l}", name=f"adw{l}")
                  for l in range(3)]
        b_sb = [sbc.tile([P, P], F32, tag=f"bb{l}", name=f"bb{l}")
                for l in range(2)]
        b3_sb = sbc.tile([P, HID], F32)
        lw_sb = sbc.tile([P, HID], F32)
        for l in range(3):
            nc.sync.dma_start(wt_sb[l][:], WT[l][:])
            nc.sync.dma_start(as_sb[l][:], asrep[l][:])
            nc.sync.dma_start(adw_sb[l][:], adrep[l][:])
        for l in range(2):
            nc.sync.dma_start(b_sb[l][:], brep[l][:])
        nc.sync.dma_start(b3_sb[:], b3rep[:])
        nc.sync.dma_start(lw_sb[:], lwrep[:])

        def w_transform_x(q, rb):
            pos = q * RB * 128
            end = pos + rb * 128
            while pos < end:
                w = min(512, end - pos)
                xs = sb.tile([P, 512], F16, tag="xs")
                nc.sync.dma_start(xs[:, :w], xT[:, pos:pos + w])
                hps = ps.tile([P, 512], F32, tag="big", space="PSUM")
                nc.tensor.matmul(hps[:, :w], lhsT=wt_sb[0][:],
                                 rhs=xs[:, :w], start=True, stop=True)
                hs = sb.tile([P, 512], F16, tag="hstage")
                nc.scalar.copy(hs[:, :w], hps[:, :w])
                nc.sync.dma_start(hT[:, pos:pos + w], hs[:, :w])
                pos += w

        def w_transform_y(l, q, rb):
            # fused: transpose yv (4 blocks) -> ytc chunk -> W matmul -> hT
            for b0 in range(0, rb, 4):
                s0 = q * RB + b0
                nb = min(4, rb - b0)
                ytc = sb.tile([P, 512], F16, tag="ytc")
                for t in range(nb):
                    tp = ps.tile([P, P], F32, tag="big", space="PSUM")
                    nc.tensor.transpose(out=tp[:], in_=yv[:, s0 + t, :],
                                        identity=id32_sb[:])
                    nc.scalar.copy(ytc[:, t * P:(t + 1) * P], tp[:])
                hps = ps.tile([P, 512], F32, tag="big", space="PSUM")
                nc.tensor.matmul(hps[:, :nb * P], lhsT=wt_sb[l][:],
                                 rhs=ytc[:, :nb * P], start=True, stop=True)
                hs = sb.tile([P, 512], F16, tag="hstage")
                nc.scalar.copy(hs[:, :nb * P], hps[:, :nb * P])
                nc.sync.dma_start(hT[:, s0 * P:s0 * P + nb * P],
                                  hs[:, :nb * P])

        def build_tables(l, q, rb):
            b0, r0 = q * RB, q * QROW
            nc.sync.dma_start_transpose(
                hnode[:, b0:b0 + rb, :],
                hT[:, b0 * 128:(b0 + rb) * 128])
            as_node = sb.tile([P, RB, 4], F32, tag="asred")
            ad_node = sb.tile([P, RB, 4], F32, tag="adred")
            QB = 13
            for rep, red in ((as_sb[l], as_node), (adw_sb[l], ad_node)):
                for q0 in range(0, rb, QB):
                    qn = min(QB, rb - q0)
                    tmp = sb.tile([P, QB, P], F16, tag="ashtmp", bufs=1)
                    nc.vector.tensor_tensor(
                        out=tmp[:, :qn, :], in0=hnode[:, b0 + q0:b0 + q0 + qn, :],
                        in1=rep[:].unsqueeze(1).to_broadcast([P, qn, P]),
                        op=mybir.AluOpType.mult)
                    nc.vector.tensor_reduce(
                        out=red[:, q0:q0 + qn, :],
                        in_=tmp[:, :qn, :].rearrange("p s (h c) -> p s h c", h=4),
                        axis=mybir.AxisListType.X, op=mybir.AluOpType.add)
            nc.vector.tensor_copy(ad_sb[:, b0:b0 + rb, :], ad_node[:, :rb, :])
            # quarter-table writes (h, as, pad blocks)
            nc.sync.dma_start(
                tbl_in[r0:r0 + rb * 128, 0:P].rearrange(
                    "(s p) f -> p s f", p=P),
                hnode[:, b0:b0 + rb, :])
            nc.sync.dma_start(
                tbl_in[:].bitcast(F32)[r0:r0 + rb * 128, 64:68].rearrange(
                    "(s p) f -> p s f", p=P),
                as_node[:, :rb, :])
            for t in range(rb, RB + 1):
                nc.sync.dma_start(
                    tbl_in[r0 + t * 128:r0 + (t + 1) * 128, :].rearrange(
                        "(s p) f -> p s f", p=P),
                    padblk[:].unsqueeze(1))
            # self-loop term: init acc with exp(lrelu(as+ad)) weighted h
            es = sb.tile([P, RB, 4], F32, tag="es")
            nc.vector.tensor_tensor(out=es[:, :rb, :], in0=as_node[:, :rb, :],
                                    in1=ad_node[:, :rb, :],
                                    op=mybir.AluOpType.add)
            nc.vector.scalar_tensor_tensor(
                out=es[:, :rb, :], in0=es[:, :rb, :], scalar=NEG,
                in1=es[:, :rb, :],
                op0=mybir.AluOpType.mult, op1=mybir.AluOpType.max)
            exs = sb.tile([P, RB, 4], F16, tag="exs")
            nc.scalar.activation(exs[:, :rb, :], es[:, :rb, :],
                                 mybir.ActivationFunctionType.Exp)
            nc.vector.tensor_copy(acc[:, b0:b0 + rb, 128:132], exs[:, :rb, :])
            nc.vector.tensor_tensor(
                out=yv[:, b0:b0 + rb, :].rearrange("p s (h c) -> p s h c", h=4),
                in0=hnode[:, b0:b0 + rb, :].rearrange(
                    "p s (h c) -> p s h c", h=4),
                in1=exs[:, :rb, :].unsqueeze(3).to_broadcast([P, rb, 4, 32]),
                op=mybir.AluOpType.mult)

        def collective(q, tq):
            if "noag" in variant:
                return
            nc.gpsimd.collective_compute(
                "AllGather", mybir.AluOpType.bypass,
                replica_groups=[list(range(NCORES))],
                ins=[tbl_in[q * QROW:(q + 1) * QROW, :].opt()],
                outs=[tq[q][:].opt()])

        def run_edges(tq, pro_cbs=()):
            # pro_cbs: list of (trigger_seg, ag_seg, fn, ag_fn); fn fires
            # after the trigger segment's scatters are emitted, ag_fn a few
            # segments later (keeps the collective off the Pool queue head
            # until its inputs are nearly ready)
            grp_ps = {}
            prev = None
            pending = sorted(pro_cbs, key=lambda t: t[0])
            scattered = -1
            for segi, seg in enumerate(list(segs) + [None]):
                if seg is not None:
                    c0, nch, p0, npar = seg
                    g = chunk_parts[c0][0][0]
                    msgs = sb.tile([P, SEGC, 256], F16, tag="msgs")
                    idx_sb = sb.tile([P, SEGC * 8], I16, tag="idxseg")
                    nc.sync.dma_start(idx_sb[:, :nch * 8],
                                      idx16[:, c0 * 8:(c0 + nch) * 8])
                    d8_sb = sb.tile([P, SEGP], F32, tag="d8seg")
                    nc.sync.dma_start(d8_sb[:, :npar],
                                      d8col[:, p0:p0 + npar])
                    mT8 = sb.tile([P, SEGP, P], F8, tag="mT8seg")
                    nc.sync.dma_start(
                        mT8[:, :npar, :],
                        maskt[:, p0 * 128:(p0 + npar) * 128])
                    mT = sb.tile([P, SEGP, P], F16, tag="mTseg")
                    nc.scalar.copy(mT[:, :npar, :], mT8[:, :npar, :])
                    if "nogather" not in variant:
                        # split across the 4 SWDGE queues: 4 rings per SDMA
                        # engine keep more HBM reads in flight (~2x faster
                        # than one ring for 512B random rows)
                        a = 0
                        qn = segi % 4
                        while a < nch:
                            b = min(a + 6, nch)
                            nidx = (b - a) * 128
                            nc.gpsimd.dma_gather(
                                msgs[:, a:b, :], tq[g][:],
                                idx_sb[:, a * 8:b * 8],
                                nidx, nidx, 256, single_packet=False,
                                queue_num=qn)
                            qn = (qn + 1) % 4
                            a = b
                    maskS = sb.tile([P, SEGP, P], F16, tag="maskS", bufs=3)
                    if "nomask" not in variant:
                        for pp in range(npar):
                            nc.vector.tensor_scalar(
                                out=maskS[:, pp, :], in0=iota_sb[:],
                                scalar1=d8_sb[:, pp:pp + 1], scalar2=None,
                                op0=mybir.AluOpType.is_equal)
                    ad_ps = ps.tile([P, SEGC, 4], F32, tag="adps",
                                    space="PSUM")
                    if "nope" not in variant:
                        pp = 0
                        for k in range(nch):
                            parts = chunk_parts[c0 + k]
                            for t, (g2, j, lo, hi, fi, la) in enumerate(parts):
                                nc.tensor.matmul(
                                    ad_ps[:, k, :], lhsT=mT[:, pp, :],
                                    rhs=ad_sb[:, j, :], start=(t == 0),
                                    stop=(t == len(parts) - 1))
                                pp += 1
                    wmsg = sb.tile([P, SEGC, 132], F16, tag="wmsg")
                    e1 = sb.tile([P, SEGC, 4], F32, tag="e1")
                    nc.vector.tensor_tensor(
                        out=e1[:, :nch, :],
                        in0=msgs[:].bitcast(F32)[:, :nch, 64:68],
                        in1=ad_ps[:, :nch, :], op=mybir.AluOpType.add)
                    nc.vector.scalar_tensor_tensor(
                        out=e1[:, :nch, :], in0=e1[:, :nch, :], scalar=NEG,
                        in1=e1[:, :nch, :],
                        op0=mybir.AluOpType.mult, op1=mybir.AluOpType.max)
                    nc.scalar.activation(wmsg[:, :nch, 128:132],
                                         e1[:, :nch, :],
                                         mybir.ActivationFunctionType.Exp)
                    nc.vector.tensor_tensor(
                        out=wmsg[:, :nch, 0:128].rearrange(
                            "p s (h c) -> p s h c", h=4),
                        in0=msgs[:, :nch, 0:128].rearrange(
                            "p s (h c) -> p s h c", h=4),
                        in1=wmsg[:, :nch, 128:132].unsqueeze(3).to_broadcast(
                            [P, nch, 4, 32]),
                        op=mybir.AluOpType.mult)
                    cur = (c0, nch, maskS, wmsg)
                else:
                    cur = None
                # previous segment's scatter matmuls (PE overlaps the
                # exp/weight chain above with these accumulations)
                if prev is not None and "nope" not in variant:
                    pc0, pnch, pmask, pwmsg = prev
                    pp = 0
                    for k in range(pnch):
                        for (g2, j, lo, hi, fi, la) in chunk_parts[pc0 + k]:
                            key = (g2, j)
                            if fi:
                                grp_ps[key] = ps.tile([P, 132], F32,
                                                      tag="grp", name="grp",
                                                      space="PSUM")
                            gp = grp_ps[key]
                            nc.tensor.matmul(gp[:], lhsT=pmask[:, pp, :],
                                             rhs=pwmsg[:, k, :],
                                             start=fi, stop=la)
                            if la:
                                nc.vector.tensor_tensor(
                                    out=acc[:, j, :], in0=acc[:, j, :],
                                    in1=gp[:], op=mybir.AluOpType.add)
                            pp += 1
                    scattered += 1
                for (ts, tag_, fn, ag_fn) in pending:
                    if fn is not None and scattered >= ts:
                        fn()
                    if ag_fn is not None and (scattered >= tag_
                                              or cur is None):
                        ag_fn()
                pending = [(ts, tag_, None if (fn is None or scattered >= ts)
                            else fn,
                            None if (ag_fn is None or scattered >= tag_
                                     or cur is None) else ag_fn)
                           for (ts, tag_, fn, ag_fn) in pending]
                pending = [t for t in pending
                           if t[2] is not None or t[3] is not None]
                prev = cur

        def elu_inplace(full_ap, nblk, width):
            EB = 4
            for q0 in range(0, nblk, EB):
                qn = min(EB, nblk - q0)
                ap = full_ap[:, q0:q0 + qn, :]
                shape = [P, EB, width]
                a = sb.tile(shape, F32, tag="elua", bufs=1)
                nc.scalar.activation(a[:, :qn, :], ap,
                                     mybir.ActivationFunctionType.Relu)
                bmin = sb.tile(shape, F32, tag="elub", bufs=1)
                nc.vector.tensor_scalar(out=bmin[:, :qn, :], in0=ap,
                                        scalar1=0.0, scalar2=None,
                                        op0=mybir.AluOpType.min)
                cc = sb.tile(shape, F32, tag="eluc", bufs=1)
                nc.scalar.activation(cc[:, :qn, :], bmin[:, :qn, :],
                                     mybir.ActivationFunctionType.Exp)
                nc.vector.scalar_tensor_tensor(
                    out=ap, in0=a[:, :qn, :], scalar=-1.0, in1=cc[:, :qn, :],
                    op0=mybir.AluOpType.add, op1=mybir.AluOpType.add)

        def finalize(l, q, rb):
            b0 = q * RB
            rec = sb.tile([P, RB, 4], F32, tag="rec")
            nc.vector.reciprocal(out=rec[:, :rb, :],
                                 in_=acc[:, b0:b0 + rb, 128:132])
            nc.vector.tensor_tensor(
                out=yv[:, b0:b0 + rb, :].rearrange("p s (h c) -> p s h c", h=4),
                in0=acc[:, b0:b0 + rb, 0:128].rearrange(
                    "p s (h c) -> p s h c", h=4),
                in1=rec[:, :rb, :].unsqueeze(3).to_broadcast([P, rb, 4, 32]),
                op=mybir.AluOpType.mult)
            if l < 2:
                nc.vector.tensor_tensor(
                    out=yv[:, b0:b0 + rb, :], in0=yv[:, b0:b0 + rb, :],
                    in1=b_sb[l][:].unsqueeze(1).to_broadcast([P, rb, P]),
                    op=mybir.AluOpType.add)
                elu_inplace(yv[:, b0:b0 + rb, :], rb, P)

        def make_prologue(l, q, tq, first):
            rb = min(RB, NBLK - q * RB)

            def fn():
                if not first:
                    finalize(2 if l == 0 else l - 1, q, rb)
                if l == 0:
                    w_transform_x(q, rb)
                else:
                    w_transform_y(l, q, rb)
                build_tables(l, q, rb)

            def ag_fn():
                collective(q, tq)
            return fn, ag_fn

        nseg_all = len(segs)
        phases = [(r, l) for r in range(repeat) for l in range(3)]
        tqs = []
        for (r, l) in phases:
            tqs.append([dr.tile([GSZ, 256], F16, addr_space="Shared",
                                name=f"tq{r}_{l}_{q}", tag=f"tq{r}_{l}_{q}")
                        for q in range(4)])
        # phase 0's prologue emitted inline; phase i+1's inside run_edges(i)
        for q in range(4):
            fn, ag_fn = make_prologue(0, q, tqs[0], True)
            fn(); ag_fn()
        for i, (r, l) in enumerate(phases):
            cbs = []
            if i + 1 < len(phases):
                nl = phases[i + 1][1]
                for q in range(4):
                    fn, ag_fn = make_prologue(nl, q, tqs[i + 1], False)
                    cbs.append((triggers[q],
                                min(triggers[q] + AGAP, nseg_all - 1), fn,
                                ag_fn))
            run_edges(tqs[i], cbs)
        finalize(2, 0, RB); finalize(2, 1, RB); finalize(2, 2, RB)
        finalize(2, 3, NBLK - 3 * RB)

        h3 = sb.tile([P, NBLK, HID], F32, tag="h3", bufs=1)
        nc.vector.tensor_reduce(
            out=h3[:], in_=yv[:].rearrange("p s (h c) -> p s c h", h=4),
            axis=mybir.AxisListType.X, op=mybir.AluOpType.add)
        nc.vector.tensor_scalar(out=h3[:], in0=h3[:], scalar1=0.25,
                                scalar2=None, op0=mybir.AluOpType.mult)
        nc.vector.tensor_tensor(
            out=h3[:], in0=h3[:],
            in1=b3_sb[:].unsqueeze(1).to_broadcast([P, NBLK, HID]),
            op=mybir.AluOpType.add)
        elu_inplace(h3, NBLK, HID)
        pv = sb.tile([P, NBLK], F32, tag="pv", bufs=1)
        for q0 in range(0, NBLK, 16):
            qn = min(16, NBLK - q0)
            tmp3 = sb.tile([P, 16, HID], F32, tag="tmp3", bufs=1)
            nc.vector.tensor_tensor(
                out=tmp3[:, :qn, :], in0=h3[:, q0:q0 + qn, :],
                in1=lw_sb[:].unsqueeze(1).to_broadcast([P, qn, HID]),
                op=mybir.AluOpType.mult)
            nc.vector.tensor_reduce(out=pv[:, q0:q0 + qn], in_=tmp3[:, :qn, :],
                                    axis=mybir.AxisListType.X,
                                    op=mybir.AluOpType.add)
        pool_ps = ps.tile([64, 1], F32, tag="big", space="PSUM")
        for s in range(NBLK):
            bps = sb.tile([P, 64], F32, tag="bps")
            nc.sync.dma_start(bps[:], bpool[:, s * 64:(s + 1) * 64])
            nc.tensor.matmul(pool_ps[:], lhsT=bps[:], rhs=pv[:, s:s + 1],
                             start=(s == 0), stop=(s == NBLK - 1))
        pool_sb = sb.tile([64, 1], F32, tag="poolsb", bufs=1)
        nc.scalar.copy(pool_sb[:], pool_ps[:])
        nc.sync.dma_start(out64[:], pool_sb[:])

    nc.compile()
    return nc


# ----------------------------------------------------------------------------
# host-side input construction
# ----------------------------------------------------------------------------
def make_inputs(sched, idx_arrs, d8_arrs, mt_arrs, inputs, batch_counts=None):
    """Per-core in_maps from the raw problem inputs dict."""
    SH, NBLK = sched["SH"], sched["NBLK"]
    NROW = NBLK * 128
    x = np.asarray(inputs["x"], np.float32)
    N = x.shape[0]
    batch = np.asarray(inputs["batch"], np.int64)
    NGr = 64 if batch_counts is None else len(batch_counts)
    counts = np.bincount(batch, minlength=NGr).astype(np.float32)
    counts[counts == 0] = 1.0

    def rep(v, dt=np.float32):
        v = np.asarray(v, np.float32).reshape(1, -1)
        return np.tile(v, (P, 1)).astype(dt)

    Ws = [np.asarray(inputs[k], np.float32).T.astype(np.float16).copy()
          for k in ("W1", "W2", "W3")]
    asr = [rep(np.asarray(inputs[k], np.float32).reshape(-1), np.float16)
           for k in ("a1s", "a2s", "a3s")]
    adr = [rep(np.asarray(inputs[k], np.float32).reshape(-1), np.float16)
           for k in ("a1d", "a2d", "a3d")]
    br = [rep(inputs["b1"]), rep(inputs["b2"])]
    b3r = rep(inputs["b3"])
    lwr = rep(np.asarray(inputs["lin_w"], np.float32).reshape(-1))
    iot = np.tile(np.arange(P, dtype=np.float32), (P, 1)).astype(np.float16)
    idf32 = np.eye(P, dtype=np.float32)
    pad = np.zeros((P, 256), np.float16)
    pad.view(np.float32)[:, 64:68] = PAD_AS

    in_maps = []
    for c in range(NCORES):
        xs = np.zeros((NROW, P), np.float32)
        xs[0:SH] = x[c * SH:(c + 1) * SH]
        bp = np.zeros((NROW, 64), np.float32)
        b_loc = batch[c * SH:(c + 1) * SH]
        bp[np.arange(SH), b_loc] = 1.0 / counts[b_loc]
        m = {"xT": np.ascontiguousarray(xs.T).astype(np.float16),
             "idx16": idx_arrs[c], "d8col": d8_arrs[c], "maskt": mt_arrs[c],
             "b3rep": b3r, "lwrep": lwr, "iotarep": iot,
             "ident32": idf32, "padblk": pad,
             "bpool": np.ascontiguousarray(
                 bp.reshape(NBLK, P, 64).transpose(1, 0, 2).reshape(
                     P, NBLK * 64))}
        for l in range(3):
            m[f"WT{l}"] = Ws[l]
            m[f"asrep{l}"] = asr[l]
            m[f"adrep{l}"] = adr[l]
        for l in range(2):
            m[f"brep{l}"] = br[l]
        in_maps.append(m)
    return in_maps


# ----------------------------------------------------------------------------
# SPMD runner (modeled on bass2jax.run_bass_via_pjrt, with reusable executable)
# ----------------------------------------------------------------------------
def make_runner(nc, in_maps):
    import jax
    import jax.numpy as jnp
    from jax.sharding import Mesh, PartitionSpec
    from jax.experimental.shard_map import shard_map
    from concourse import bass2jax, mybir as mb

    bass2jax.install_neuronx_cc_hook()
    n_cores = len(in_maps)
    part_name = nc.partition_id_tensor.name if nc.partition_id_tensor else None
    in_names, out_names, out_avals, zero_outs = [], [], [], []
    for alloc in nc.m.functions[0].allocations:
        if not isinstance(alloc, mb.MemoryLocationSet):
            continue
        name = alloc.memorylocations[0].name
        if alloc.kind == "ExternalInput":
            if name != part_name:
                in_names.append(name)
        elif alloc.kind == "ExternalOutput":
            out_names.append(name)
            shape = tuple(alloc.tensor_shape)
            dtype = mb.dt.np(alloc.dtype)
            out_avals.append(jax.core.ShapedArray(shape, dtype))
            zero_outs.append(np.zeros(shape, dtype))
    n_params = len(in_names)
    all_names = in_names + out_names
    if part_name is not None:
        all_names = all_names + [part_name]

    def _body(*args):
        operands = list(args)
        if part_name is not None:
            operands.append(bass2jax.partition_id_tensor())
        outs = bass2jax._bass_exec_p.bind(
            *operands, out_avals=tuple(out_avals), in_names=tuple(all_names),
            out_names=tuple(out_names), lowering_input_output_aliases=(),
            sim_require_finite=False, sim_require_nnan=False, nc=nc)
        return tuple(outs)

    devices = jax.devices()[:n_cores]
    mesh = Mesh(np.asarray(devices), ("core",))
    in_specs = (PartitionSpec("core"),) * (n_params + len(out_names))
    out_specs = (PartitionSpec("core"),) * len(out_names)
    fn = jax.jit(shard_map(_body, mesh=mesh, in_specs=in_specs,
                           out_specs=out_specs, check_rep=False))
    concat_in = [np.concatenate([np.asarray(in_maps[c][nm])
                                 for c in range(n_cores)], axis=0)
                 for nm in in_names]
    concat_zeros = [np.zeros((n_cores * z.shape[0], *z.shape[1:]), z.dtype)
                    for z in zero_outs]
    dev_in = [jax.device_put(
        a, jax.sharding.NamedSharding(mesh, PartitionSpec("core")))
        for a in concat_in + concat_zeros]

    def run():
        outs = fn(*dev_in)
        outs = [np.asarray(o) for o in outs]
        return [
            {nm: outs[i].reshape(n_cores, *out_avals[i].shape)[c]
             for i, nm in enumerate(out_names)}
            for c in range(n_cores)]
    return run


def kernel(**inputs):
    """Full-input distributed GAT kernel; returns pooled [64] float32."""
    inputs = {k: np.asarray(v) for k, v in inputs.items()}
    N = inputs["x"].shape[0]
    sched, idx_arrs, d8_arrs, mt_arrs = preprocess(inputs["edge_index"], N)
    nc = build_program(sched)
    in_maps = make_inputs(sched, idx_arrs, d8_arrs, mt_arrs, inputs)
    run = make_runner(nc, in_maps)
    kernel.last_runner = run          # exposed for test.py timing
    kernel.last_inputs = inputs
    results = run()
    partial = sum(r["out64"][:, 0] for r in results)
    out = (partial + np.float32(inputs["lin_b"].reshape(-1)[0]))[:64]
    return out.astype(np.float32)



# revision 7
# speedup vs baseline: 1.0820x; 1.0461x over previous
"""Distributed GAT kernel for Trainium2 (8 NeuronCores), Bass/Tile. v2.

Architecture (per layer):
  - node tables [TROW, 256] f16 rows = [h(128 f16) | as(4 f32 as 8 f16) | pad]
    split into 4 quarter-tables; each quarter AllGather'd separately (Shared
    outputs) and triggered just before the edge pass that consumes it, so
    collectives overlap edge compute.
  - each core owns a dst shard; edges grouped by (src-quarter g, dst block j
    of 128 local dsts), chunked into 128-edge chunks (count = cross-core max).
  - per segment (24 chunks): dma_gather 512B rows by src; per chunk the
    TRANSPOSED one-hot dst mask (static!) is streamed from DRAM and the plain
    mask built by one DVE is_equal; ad via maskT matmul from SBUF ad table
    into one PSUM tile; batched e=lrelu(as+ad), ex=exp(e), weight msgs by ex;
    per chunk reduce matmul lhsT=mask rhs=[msgs|ex] accumulated per (g,j).
  - self-loop term computed densely from the local node table (initializes
    the accumulators), so self edges are not in the edge stream.
  - finalize: normalize by denom, +bias, ELU, transform with next W (f16).
  - layer 3: mean heads, +b3, ELU, dot lin_w, pool via Bpool matmul -> [64]
    partial per core; host sums partials (+lin_b).
"""
import numpy as np
import ml_dtypes
from contextlib import ExitStack

import concourse.bacc as bacc
import concourse.bass as bass
import concourse.tile as tile
from concourse import mybir, bass_utils
from concourse.library_config import mlp

F16 = mybir.dt.float16
F32 = mybir.dt.float32
F8 = mybir.dt.float8e4
I16 = mybir.dt.int16
NCORES = 8
P = 128
SEGC = 24          # chunks per gather segment
HEADS = 4
HID = 32
D1 = 128
NEG = 0.2
PAD_AS = -200.0
RB = 25            # real dst-blocks per table quarter
QROW = (RB + 1) * 128   # rows per quarter (+1 pad block) = 3328
TROW = 4 * QROW         # per-core table rows = 13312
GSZ = NCORES * QROW     # sub-table rows per quarter group = 26624
PADIDX = RB * 128       # pad row (core 0's pad block) within any sub-table


# ----------------------------------------------------------------------------
# host preprocessing
# ----------------------------------------------------------------------------
def preprocess(edge_index, N):
    """Build the core-independent schedule + per-core index/mask arrays.

    Chunks are 128 gathered edge-slots; a chunk may contain edges of several
    dst blocks ("parts"). Per part, transposed/plain one-hot masks select that
    part's edges (other slots 0), so per-(g,j) slot counts need no 128-ceil
    padding and segments need no dummy chunks.
    """
    SH = N // NCORES
    assert SH * NCORES == N
    NBLK = (SH + 127) // 128
    assert NBLK <= 4 * RB
    assert GSZ <= 32768

    src = edge_index[0].astype(np.int64)
    dst = edge_index[1].astype(np.int64)

    per_core = []
    cnt = np.zeros((NCORES, 4, NBLK), np.int64)
    for c in range(NCORES):
        m = (dst // SH) == c
        s_c, d_c = src[m], dst[m] - c * SH
        cs, i = s_c // SH, s_c % SH
        b, p = i // 128, i % 128
        g_c = b // RB
        row = cs * QROW + (b % RB) * 128 + p   # row within sub-table g
        order = np.lexsort((row, d_c, g_c))
        per_core.append((row[order], d_c[order], g_c[order]))
        np.add.at(cnt[c], (g_c[order], d_c[order] // 128), 1)

    maxcnt = cnt.max(axis=0)          # [4, NBLK] slots per (g, j) run

    # pack runs into 128-slot chunks (runs may split across chunks)
    chunk_parts = []   # per chunk: list of (g, j, lo, hi, rfirst, rlast)
    seg_bounds = []    # (c0, nch, p0, npar) per segment
    run_pos = {}       # (g, j) -> global slot start
    cbase = 0
    for g in range(4):
        pos = 0
        for j in range(NBLK):
            mc = int(maxcnt[g, j])
            if mc == 0:
                continue
            run_pos[(g, j)] = cbase * 128 + pos
            ch0, ch1 = pos // 128, (pos + mc - 1) // 128
            for ch in range(ch0, ch1 + 1):
                while cbase + ch >= len(chunk_parts):
                    chunk_parts.append([])
                lo = max(pos, ch * 128) - ch * 128
                hi = min(pos + mc, (ch + 1) * 128) - ch * 128
                chunk_parts[cbase + ch].append(
                    (g, j, lo, hi, ch == ch0, ch == ch1))
            pos += mc
        nch_pass = (pos + 127) // 128
        s = 0
        while s < nch_pass:
            nch = min(SEGC, nch_pass - s)
            seg_bounds.append((cbase + s, nch))
            s += nch
        cbase += nch_pass
    C = len(chunk_parts)

    # part program order + segment part offsets
    part_list = []     # (g, j, ci, lo, hi, rfirst, rlast)
    pmap = {}          # (g, j, ci) -> part index
    for ci, parts in enumerate(chunk_parts):
        for (g, j, lo, hi, fi, la) in parts:
            pmap[(g, j, ci)] = len(part_list)
            part_list.append((g, j, ci, lo, hi, fi, la))
    PARTS = len(part_list)
    segs = []
    for (c0, nch) in seg_bounds:
        p0 = pmap[chunk_parts[c0][0][:2] + (c0,)]
        npar = sum(len(chunk_parts[c]) for c in range(c0, c0 + nch))
        segs.append((c0, nch, p0, npar))
    SEGP = max(np_ for _, _, _, np_ in segs)

    idx_arrs, d8_arrs, mt_arrs = [], [], []
    for c in range(NCORES):
        s_row, d_l, g_c = per_core[c]
        idx = np.full((C, 128), PADIDX, np.int64)
        d8p = np.full((PARTS, 128), 255, np.int64)
        for g in range(4):
            mg = g_c == g
            sg, dg = s_row[mg], d_l[mg]
            blocks = dg // 128
            for j in np.unique(blocks):
                mb = blocks == j
                rows, dl = sg[mb], dg[mb]
                S0 = run_pos[(g, int(j))]
                n = len(rows)
                gs = S0 + np.arange(n)
                cis, sic = gs // 128, gs % 128
                idx[cis, sic] = rows
                ch0 = S0 // 128
                ch1 = (S0 + int(maxcnt[g, j]) - 1) // 128
                pis_by_ci = np.array(
                    [pmap[(g, int(j), ci)] for ci in range(ch0, ch1 + 1)])
                d8p[pis_by_ci[cis - ch0], sic] = dl - int(j) * 128
        # dma_gather layout: idx i -> partition i%16, col i//16; replicate x8
        flat = idx.reshape(-1)
        il = np.zeros((16, C * 8), np.int16)
        ar = np.arange(C * 128)
        il[ar % 16, ar // 16] = flat.astype(np.int16)
        idx_arrs.append(np.tile(il, (8, 1)))
        d8_arrs.append(np.ascontiguousarray(d8p.T).astype(np.float32))
        mt = (np.arange(128)[:, None, None] == d8p[None, :, :]
              ).astype(ml_dtypes.float8_e4m3)
        mt_arrs.append(np.ascontiguousarray(mt.reshape(128, PARTS * 128)))
    # per quarter q: first segment index by whose end all last-pass (g=3)
    # retires of quarter q's blocks have been emitted
    last_seg_of_run = {}
    for si, (c0, nch, p0, npar) in enumerate(segs):
        for ci in range(c0, c0 + nch):
            for (g, j, lo, hi, fi, la) in chunk_parts[ci]:
                if la:
                    last_seg_of_run[(g, j)] = si
    triggers = []
    for q in range(4):
        rb = min(RB, NBLK - q * RB)
        t = 0
        for j in range(q * RB, q * RB + rb):
            if (3, j) in last_seg_of_run:
                t = max(t, last_seg_of_run[(3, j)])
        triggers.append(t)
    sched = dict(SH=SH, NBLK=NBLK, C=C, PARTS=PARTS, SEGP=SEGP,
                 chunk_parts=chunk_parts, segs=segs, triggers=triggers)
    return sched, idx_arrs, d8_arrs, mt_arrs


# ----------------------------------------------------------------------------
# device program
# ----------------------------------------------------------------------------
def build_program(sched, repeat=1, variant=()):
    SH, NBLK, C = sched["SH"], sched["NBLK"], sched["C"]
    PARTS, SEGP = sched["PARTS"], sched["SEGP"]
    chunk_parts, segs = sched["chunk_parts"], sched["segs"]
    triggers = sched["triggers"]
    NROW = NBLK * 128

    nc = bacc.Bacc("TRN2", target_bir_lowering=False, debug=False,
                   num_devices=NCORES, num_swdge_queues=4)

    xT = nc.dram_tensor("xT", [P, NROW], F16, kind="ExternalInput")
    idx16 = nc.dram_tensor("idx16", [P, C * 8], I16, kind="ExternalInput")
    d8col = nc.dram_tensor("d8col", [P, PARTS], F32, kind="ExternalInput")
    maskt = nc.dram_tensor("maskt", [P, PARTS * 128], F8,
                           kind="ExternalInput")
    WT = [nc.dram_tensor(f"WT{l}", [P, P], F16, kind="ExternalInput")
          for l in range(3)]
    asrep = [nc.dram_tensor(f"asrep{l}", [P, P], F16, kind="ExternalInput")
             for l in range(3)]
    adrep = [nc.dram_tensor(f"adrep{l}", [P, P], F16, kind="ExternalInput")
             for l in range(3)]
    brep = [nc.dram_tensor(f"brep{l}", [P, P], F32, kind="ExternalInput")
            for l in range(2)]
    b3rep = nc.dram_tensor("b3rep", [P, HID], F32, kind="ExternalInput")
    lwrep = nc.dram_tensor("lwrep", [P, HID], F32, kind="ExternalInput")
    iotarep = nc.dram_tensor("iotarep", [P, P], F16, kind="ExternalInput")
    ident32 = nc.dram_tensor("ident32", [P, P], F32, kind="ExternalInput")
    padblk = nc.dram_tensor("padblk", [P, 256], F16, kind="ExternalInput")
    bpool = nc.dram_tensor("bpool", [P, NBLK * 64], F32, kind="ExternalInput")
    out64 = nc.dram_tensor("out64", [64, 1], F32, kind="ExternalOutput")

    with tile.TileContext(nc) as tc, ExitStack() as ctx:
        sb = ctx.enter_context(tc.tile_pool(name="sb", bufs=2))
        sbc = ctx.enter_context(tc.tile_pool(name="sbc", bufs=1))
        ps = ctx.enter_context(tc.tile_pool(name="ps", bufs=2, space="PSUM"))
        dr = ctx.enter_context(tc.tile_pool(name="dr", bufs=1, space="DRAM"))

        nc.gpsimd.load_library(mlp)

        iota_sb = sbc.tile([P, P], F16)
        nc.sync.dma_start(iota_sb[:], iotarep[:])
        id32_sb = sbc.tile([P, P], F32)
        nc.sync.dma_start(id32_sb[:], ident32[:])
        acc = sbc.tile([P, NBLK, 132], F32)
        ad_sb = sbc.tile([P, NBLK, 4], F16)
        hnode = sbc.tile([P, NBLK, P], F16)
        yv = acc[:, :, 0:128]

        tbl_in = dr.tile([TROW, 256], F16)
        hT = dr.tile([P, NROW], F16, name="hTd")

        wt_sb = [sbc.tile([P, P], F16, tag=f"wt{l}", name=f"wt{l}")
                 for l in range(3)]
        as_sb = [sbc.tile([P, P], F16, tag=f"asw{l}", name=f"asw{l}")
                 for l in range(3)]
        adw_sb = [sbc.tile([P, P], F16, tag=f"adw{l}", name=f"adw{l}")
                  for l in range(3)]
        b_sb = [sbc.tile([P, P], F32, tag=f"bb{l}", name=f"bb{l}")
                for l in range(2)]
        b3_sb = sbc.tile([P, HID], F32)
        lw_sb = sbc.tile([P, HID], F32)
        for l in range(3):
            nc.sync.dma_start(wt_sb[l][:], WT[l][:])
            nc.sync.dma_start(as_sb[l][:], asrep[l][:])
            nc.sync.dma_start(adw_sb[l][:], adrep[l][:])
        for l in range(2):
            nc.sync.dma_start(b_sb[l][:], brep[l][:])
        nc.sync.dma_start(b3_sb[:], b3rep[:])
        nc.sync.dma_start(lw_sb[:], lwrep[:])

        def w_transform_x(q, rb):
            pos = q * RB * 128
            end = pos + rb * 128
            while pos < end:
                w = min(512, end - pos)
                xs = sb.tile([P, 512], F16, tag="xs")
                nc.sync.dma_start(xs[:, :w], xT[:, pos:pos + w])
                hps = ps.tile([P, 512], F32, tag="big", space="PSUM")
                nc.tensor.matmul(hps[:, :w], lhsT=wt_sb[0][:],
                                 rhs=xs[:, :w], start=True, stop=True)
                hs = sb.tile([P, 512], F16, tag="hstage")
                nc.scalar.copy(hs[:, :w], hps[:, :w])
                nc.sync.dma_start(hT[:, pos:pos + w], hs[:, :w])
                pos += w

        def w_transform_y(l, q, rb):
            # fused: transpose yv (4 blocks) -> ytc chunk -> W matmul -> hT
            for b0 in range(0, rb, 4):
                s0 = q * RB + b0
                nb = min(4, rb - b0)
                ytc = sb.tile([P, 512], F16, tag="ytc")
                for t in range(nb):
                    tp = ps.tile([P, P], F32, tag="big", space="PSUM")
                    nc.tensor.transpose(out=tp[:], in_=yv[:, s0 + t, :],
                                        identity=id32_sb[:])
                    nc.scalar.copy(ytc[:, t * P:(t + 1) * P], tp[:])
                hps = ps.tile([P, 512], F32, tag="big", space="PSUM")
                nc.tensor.matmul(hps[:, :nb * P], lhsT=wt_sb[l][:],
                                 rhs=ytc[:, :nb * P], start=True, stop=True)
                hs = sb.tile([P, 512], F16, tag="hstage")
                nc.scalar.copy(hs[:, :nb * P], hps[:, :nb * P])
                nc.sync.dma_start(hT[:, s0 * P:s0 * P + nb * P],
                                  hs[:, :nb * P])

        def build_tables(l, q, rb):
            b0, r0 = q * RB, q * QROW
            nc.sync.dma_start_transpose(
                hnode[:, b0:b0 + rb, :],
                hT[:, b0 * 128:(b0 + rb) * 128])
            as_node = sb.tile([P, RB, 4], F32, tag="asred")
            ad_node = sb.tile([P, RB, 4], F32, tag="adred")
            QB = 13
            for rep, red in ((as_sb[l], as_node), (adw_sb[l], ad_node)):
                for q0 in range(0, rb, QB):
                    qn = min(QB, rb - q0)
                    tmp = sb.tile([P, QB, P], F16, tag="ashtmp", bufs=1)
                    nc.vector.tensor_tensor(
                        out=tmp[:, :qn, :], in0=hnode[:, b0 + q0:b0 + q0 + qn, :],
                        in1=rep[:].unsqueeze(1).to_broadcast([P, qn, P]),
                        op=mybir.AluOpType.mult)
                    nc.vector.tensor_reduce(
                        out=red[:, q0:q0 + qn, :],
                        in_=tmp[:, :qn, :].rearrange("p s (h c) -> p s h c", h=4),
                        axis=mybir.AxisListType.X, op=mybir.AluOpType.add)
            nc.vector.tensor_copy(ad_sb[:, b0:b0 + rb, :], ad_node[:, :rb, :])
            # quarter-table writes (h, as, pad blocks)
            nc.sync.dma_start(
                tbl_in[r0:r0 + rb * 128, 0:P].rearrange(
                    "(s p) f -> p s f", p=P),
                hnode[:, b0:b0 + rb, :])
            nc.sync.dma_start(
                tbl_in[:].bitcast(F32)[r0:r0 + rb * 128, 64:68].rearrange(
                    "(s p) f -> p s f", p=P),
                as_node[:, :rb, :])
            for t in range(rb, RB + 1):
                nc.sync.dma_start(
                    tbl_in[r0 + t * 128:r0 + (t + 1) * 128, :].rearrange(
                        "(s p) f -> p s f", p=P),
                    padblk[:].unsqueeze(1))
            # self-loop term: init acc with exp(lrelu(as+ad)) weighted h
            es = sb.tile([P, RB, 4], F32, tag="es")
            nc.vector.tensor_tensor(out=es[:, :rb, :], in0=as_node[:, :rb, :],
                                    in1=ad_node[:, :rb, :],
                                    op=mybir.AluOpType.add)
            nc.vector.scalar_tensor_tensor(
                out=es[:, :rb, :], in0=es[:, :rb, :], scalar=NEG,
                in1=es[:, :rb, :],
                op0=mybir.AluOpType.mult, op1=mybir.AluOpType.max)
            exs = sb.tile([P, RB, 4], F16, tag="exs")
            nc.scalar.activation(exs[:, :rb, :], es[:, :rb, :],
                                 mybir.ActivationFunctionType.Exp)
            nc.vector.tensor_copy(acc[:, b0:b0 + rb, 128:132], exs[:, :rb, :])
            nc.vector.tensor_tensor(
                out=yv[:, b0:b0 + rb, :].rearrange("p s (h c) -> p s h c", h=4),
                in0=hnode[:, b0:b0 + rb, :].rearrange(
                    "p s (h c) -> p s h c", h=4),
                in1=exs[:, :rb, :].unsqueeze(3).to_broadcast([P, rb, 4, 32]),
                op=mybir.AluOpType.mult)

        def collective(q, tq):
            if "noag" in variant:
                return
            nc.gpsimd.collective_compute(
                "AllGather", mybir.AluOpType.bypass,
                replica_groups=[list(range(NCORES))],
                ins=[tbl_in[q * QROW:(q + 1) * QROW, :].opt()],
                outs=[tq[q][:].opt()])

        def run_edges(tq, pro_cbs=()):
            # pro_cbs: list of (trigger_seg, ag_seg, fn, ag_fn); fn fires
            # after the trigger segment's scatters are emitted, ag_fn a few
            # segments later (keeps the collective off the Pool queue head
            # until its inputs are nearly ready)
            grp_ps = {}
            prev = None
            pending = sorted(pro_cbs, key=lambda t: t[0])
            scattered = -1
            for seg in list(segs) + [None]:
                if seg is not None:
                    c0, nch, p0, npar = seg
                    g = chunk_parts[c0][0][0]
                    msgs = sb.tile([P, SEGC, 256], F16, tag="msgs")
                    idx_sb = sb.tile([P, SEGC * 8], I16, tag="idxseg")
                    nc.sync.dma_start(idx_sb[:, :nch * 8],
                                      idx16[:, c0 * 8:(c0 + nch) * 8])
                    d8_sb = sb.tile([P, SEGP], F32, tag="d8seg")
                    nc.sync.dma_start(d8_sb[:, :npar],
                                      d8col[:, p0:p0 + npar])
                    mT8 = sb.tile([P, SEGP, P], F8, tag="mT8seg")
                    nc.sync.dma_start(
                        mT8[:, :npar, :],
                        maskt[:, p0 * 128:(p0 + npar) * 128])
                    mT = sb.tile([P, SEGP, P], F16, tag="mTseg")
                    nc.scalar.copy(mT[:, :npar, :], mT8[:, :npar, :])
                    if "nogather" not in variant:
                        # split across the 4 SWDGE queues: 4 rings per SDMA
                        # engine keep more HBM reads in flight (~2x faster
                        # than one ring for 512B random rows)
                        a = 0
                        qn = 0
                        while a < nch:
                            b = min(a + 6, nch)
                            nidx = (b - a) * 128
                            nc.gpsimd.dma_gather(
                                msgs[:, a:b, :], tq[g][:],
                                idx_sb[:, a * 8:b * 8],
                                nidx, nidx, 256, single_packet=False,
                                queue_num=qn)
                            qn = (qn + 1) % 4
                            a = b
                    maskS = sb.tile([P, SEGP, P], F16, tag="maskS", bufs=3)
                    if "nomask" not in variant:
                        for pp in range(npar):
                            nc.vector.tensor_scalar(
                                out=maskS[:, pp, :], in0=iota_sb[:],
                                scalar1=d8_sb[:, pp:pp + 1], scalar2=None,
                                op0=mybir.AluOpType.is_equal)
                    ad_ps = ps.tile([P, SEGC, 4], F32, tag="adps",
                                    space="PSUM")
                    if "nope" not in variant:
                        pp = 0
                        for k in range(nch):
                            parts = chunk_parts[c0 + k]
                            for t, (g2, j, lo, hi, fi, la) in enumerate(parts):
                                nc.tensor.matmul(
                                    ad_ps[:, k, :], lhsT=mT[:, pp, :],
                                    rhs=ad_sb[:, j, :], start=(t == 0),
                                    stop=(t == len(parts) - 1))
                                pp += 1
                    wmsg = sb.tile([P, SEGC, 132], F16, tag="wmsg")
                    e1 = sb.tile([P, SEGC, 4], F32, tag="e1")
                    nc.vector.tensor_tensor(
                        out=e1[:, :nch, :],
                        in0=msgs[:].bitcast(F32)[:, :nch, 64:68],
                        in1=ad_ps[:, :nch, :], op=mybir.AluOpType.add)
                    nc.vector.scalar_tensor_tensor(
                        out=e1[:, :nch, :], in0=e1[:, :nch, :], scalar=NEG,
                        in1=e1[:, :nch, :],
                        op0=mybir.AluOpType.mult, op1=mybir.AluOpType.max)
                    nc.scalar.activation(wmsg[:, :nch, 128:132],
                                         e1[:, :nch, :],
                                         mybir.ActivationFunctionType.Exp)
                    nc.vector.tensor_tensor(
                        out=wmsg[:, :nch, 0:128].rearrange(
                            "p s (h c) -> p s h c", h=4),
                        in0=msgs[:, :nch, 0:128].rearrange(
                            "p s (h c) -> p s h c", h=4),
                        in1=wmsg[:, :nch, 128:132].unsqueeze(3).to_broadcast(
                            [P, nch, 4, 32]),
                        op=mybir.AluOpType.mult)
                    cur = (c0, nch, maskS, wmsg)
                else:
                    cur = None
                # previous segment's scatter matmuls (PE overlaps the
                # exp/weight chain above with these accumulations)
                if prev is not None and "nope" not in variant:
                    pc0, pnch, pmask, pwmsg = prev
                    pp = 0
                    for k in range(pnch):
                        for (g2, j, lo, hi, fi, la) in chunk_parts[pc0 + k]:
                            key = (g2, j)
                            if fi:
                                grp_ps[key] = ps.tile([P, 132], F32,
                                                      tag="grp", name="grp",
                                                      space="PSUM")
                            gp = grp_ps[key]
                            nc.tensor.matmul(gp[:], lhsT=pmask[:, pp, :],
                                             rhs=pwmsg[:, k, :],
                                             start=fi, stop=la)
                            if la:
                                nc.vector.tensor_tensor(
                                    out=acc[:, j, :], in0=acc[:, j, :],
                                    in1=gp[:], op=mybir.AluOpType.add)
                            pp += 1
                    scattered += 1
                for (ts, tag_, fn, ag_fn) in pending:
                    if fn is not None and scattered >= ts:
                        fn()
                    if ag_fn is not None and (scattered >= tag_
                                              or cur is None):
                        ag_fn()
                pending = [(ts, tag_, None if (fn is None or scattered >= ts)
                            else fn,
                            None if (ag_fn is None or scattered >= tag_
                                     or cur is None) else ag_fn)
                           for (ts, tag_, fn, ag_fn) in pending]
                pending = [t for t in pending
                           if t[2] is not None or t[3] is not None]
                prev = cur

        def elu_inplace(full_ap, nblk, width):
            EB = 4
            for q0 in range(0, nblk, EB):
                qn = min(EB, nblk - q0)
                ap = full_ap[:, q0:q0 + qn, :]
                shape = [P, EB, width]
                a = sb.tile(shape, F32, tag="elua", bufs=1)
                nc.scalar.activation(a[:, :qn, :], ap,
                                     mybir.ActivationFunctionType.Relu)
                bmin = sb.tile(shape, F32, tag="elub", bufs=1)
                nc.vector.tensor_scalar(out=bmin[:, :qn, :], in0=ap,
                                        scalar1=0.0, scalar2=None,
                                        op0=mybir.AluOpType.min)
                cc = sb.tile(shape, F32, tag="eluc", bufs=1)
                nc.scalar.activation(cc[:, :qn, :], bmin[:, :qn, :],
                                     mybir.ActivationFunctionType.Exp)
                nc.vector.scalar_tensor_tensor(
                    out=ap, in0=a[:, :qn, :], scalar=-1.0, in1=cc[:, :qn, :],
                    op0=mybir.AluOpType.add, op1=mybir.AluOpType.add)

        def finalize(l, q, rb):
            b0 = q * RB
            rec = sb.tile([P, RB, 4], F32, tag="rec")
            nc.vector.reciprocal(out=rec[:, :rb, :],
                                 in_=acc[:, b0:b0 + rb, 128:132])
            nc.vector.tensor_tensor(
                out=yv[:, b0:b0 + rb, :].rearrange("p s (h c) -> p s h c", h=4),
                in0=acc[:, b0:b0 + rb, 0:128].rearrange(
                    "p s (h c) -> p s h c", h=4),
                in1=rec[:, :rb, :].unsqueeze(3).to_broadcast([P, rb, 4, 32]),
                op=mybir.AluOpType.mult)
            if l < 2:
                nc.vector.tensor_tensor(
                    out=yv[:, b0:b0 + rb, :], in0=yv[:, b0:b0 + rb, :],
                    in1=b_sb[l][:].unsqueeze(1).to_broadcast([P, rb, P]),
                    op=mybir.AluOpType.add)
                elu_inplace(yv[:, b0:b0 + rb, :], rb, P)

        def make_prologue(l, q, tq, first):
            rb = min(RB, NBLK - q * RB)

            def fn():
                if not first:
                    finalize(2 if l == 0 else l - 1, q, rb)
                if l == 0:
                    w_transform_x(q, rb)
                else:
                    w_transform_y(l, q, rb)
                build_tables(l, q, rb)

            def ag_fn():
                collective(q, tq)
            return fn, ag_fn

        nseg_all = len(segs)
        phases = [(r, l) for r in range(repeat) for l in range(3)]
        tqs = []
        for (r, l) in phases:
            tqs.append([dr.tile([GSZ, 256], F16, addr_space="Shared",
                                name=f"tq{r}_{l}_{q}", tag=f"tq{r}_{l}_{q}")
                        for q in range(4)])
        # phase 0's prologue emitted inline; phase i+1's inside run_edges(i)
        for q in range(4):
            fn, ag_fn = make_prologue(0, q, tqs[0], True)
            fn(); ag_fn()
        for i, (r, l) in enumerate(phases):
            cbs = []
            if i + 1 < len(phases):
                nl = phases[i + 1][1]
                for q in range(4):
                    fn, ag_fn = make_prologue(nl, q, tqs[i + 1], False)
                    cbs.append((triggers[q],
                                min(triggers[q] + 4, nseg_all - 1), fn,
                                ag_fn))
            run_edges(tqs[i], cbs)
        finalize(2, 0, RB); finalize(2, 1, RB); finalize(2, 2, RB)
        finalize(2, 3, NBLK - 3 * RB)

        h3 = sb.tile([P, NBLK, HID], F32, tag="h3", bufs=1)
        nc.vector.tensor_reduce(
            out=h3[:], in_=yv[:].rearrange("p s (h c) -> p s c h", h=4),
            axis=mybir.AxisListType.X, op=mybir.AluOpType.add)
        nc.vector.tensor_scalar(out=h3[:], in0=h3[:], scalar1=0.25,
                                scalar2=None, op0=mybir.AluOpType.mult)
        nc.vector.tensor_tensor(
            out=h3[:], in0=h3[:],
            in1=b3_sb[:].unsqueeze(1).to_broadcast([P, NBLK, HID]),
            op=mybir.AluOpType.add)
        elu_inplace(h3, NBLK, HID)
        pv = sb.tile([P, NBLK], F32, tag="pv", bufs=1)
        for q0 in range(0, NBLK, 16):
            qn = min(16, NBLK - q0)
            tmp3 = sb.tile([P, 16, HID], F32, tag="tmp3", bufs=1)
            nc.vector.tensor_tensor(
                out=tmp3[:, :qn, :], in0=h3[:, q0:q0 + qn, :],
                in1=lw_sb[:].unsqueeze(1).to_broadcast([P, qn, HID]),
                op=mybir.AluOpType.mult)
            nc.vector.tensor_reduce(out=pv[:, q0:q0 + qn], in_=tmp3[:, :qn, :],
                                    axis=mybir.AxisListType.X,
                                    op=mybir.AluOpType.add)
        pool_ps = ps.tile([64, 1], F32, tag="big", space="PSUM")
        for s in range(NBLK):
            bps = sb.tile([P, 64], F32, tag="bps")
            nc.sync.dma_start(bps[:], bpool[:, s * 64:(s + 1) * 64])
            nc.tensor.matmul(pool_ps[:], lhsT=bps[:], rhs=pv[:, s:s + 1],
                             start=(s == 0), stop=(s == NBLK - 1))
        pool_sb = sb.tile([64, 1], F32, tag="poolsb", bufs=1)
        nc.scalar.copy(pool_sb[:], pool_ps[:])
        nc.sync.dma_start(out64[:], pool_sb[:])

    nc.compile()
    return nc


# ----------------------------------------------------------------------------
# host-side input construction
# ----------------------------------------------------------------------------
def make_inputs(sched, idx_arrs, d8_arrs, mt_arrs, inputs, batch_counts=None):
    """Per-core in_maps from the raw problem inputs dict."""
    SH, NBLK = sched["SH"], sched["NBLK"]
    NROW = NBLK * 128
    x = np.asarray(inputs["x"], np.float32)
    N = x.shape[0]
    batch = np.asarray(inputs["batch"], np.int64)
    NGr = 64 if batch_counts is None else len(batch_counts)
    counts = np.bincount(batch, minlength=NGr).astype(np.float32)
    counts[counts == 0] = 1.0

    def rep(v, dt=np.float32):
        v = np.asarray(v, np.float32).reshape(1, -1)
        return np.tile(v, (P, 1)).astype(dt)

    Ws = [np.asarray(inputs[k], np.float32).T.astype(np.float16).copy()
          for k in ("W1", "W2", "W3")]
    asr = [rep(np.asarray(inputs[k], np.float32).reshape(-1), np.float16)
           for k in ("a1s", "a2s", "a3s")]
    adr = [rep(np.asarray(inputs[k], np.float32).reshape(-1), np.float16)
           for k in ("a1d", "a2d", "a3d")]
    br = [rep(inputs["b1"]), rep(inputs["b2"])]
    b3r = rep(inputs["b3"])
    lwr = rep(np.asarray(inputs["lin_w"], np.float32).reshape(-1))
    iot = np.tile(np.arange(P, dtype=np.float32), (P, 1)).astype(np.float16)
    idf32 = np.eye(P, dtype=np.float32)
    pad = np.zeros((P, 256), np.float16)
    pad.view(np.float32)[:, 64:68] = PAD_AS

    in_maps = []
    for c in range(NCORES):
        xs = np.zeros((NROW, P), np.float32)
        xs[0:SH] = x[c * SH:(c + 1) * SH]
        bp = np.zeros((NROW, 64), np.float32)
        b_loc = batch[c * SH:(c + 1) * SH]
        bp[np.arange(SH), b_loc] = 1.0 / counts[b_loc]
        m = {"xT": np.ascontiguousarray(xs.T).astype(np.float16),
             "idx16": idx_arrs[c], "d8col": d8_arrs[c], "maskt": mt_arrs[c],
             "b3rep": b3r, "lwrep": lwr, "iotarep": iot,
             "ident32": idf32, "padblk": pad,
             "bpool": np.ascontiguousarray(
                 bp.reshape(NBLK, P, 64).transpose(1, 0, 2).reshape(
                     P, NBLK * 64))}
        for l in range(3):
            m[f"WT{l}"] = Ws[l]
            m[f"asrep{l}"] = asr[l]
            m[f"adrep{l}"] = adr[l]
        for l in range(2):
            m[f"brep{l}"] = br[l]
        in_maps.append(m)
    return in_maps


# ----------------------------------------------------------------------------
# SPMD runner (modeled on bass2jax.run_bass_via_pjrt, with reusable executable)
# ----------------------------------------------------------------------------
def make_runner(nc, in_maps):
    import jax
    import jax.numpy as jnp
    from jax.sharding import Mesh, PartitionSpec
    from jax.experimental.shard_map import shard_map
    from concourse import bass2jax, mybir as mb

    bass2jax.install_neuronx_cc_hook()
    n_cores = len(in_maps)
    part_name = nc.partition_id_tensor.name if nc.partition_id_tensor else None
    in_names, out_names, out_avals, zero_outs = [], [], [], []
    for alloc in nc.m.functions[0].allocations:
        if not isinstance(alloc, mb.MemoryLocationSet):
            continue
        name = alloc.memorylocations[0].name
        if alloc.kind == "ExternalInput":
            if name != part_name:
                in_names.append(name)
        elif alloc.kind == "ExternalOutput":
            out_names.append(name)
            shape = tuple(alloc.tensor_shape)
            dtype = mb.dt.np(alloc.dtype)
            out_avals.append(jax.core.ShapedArray(shape, dtype))
            zero_outs.append(np.zeros(shape, dtype))
    n_params = len(in_names)
    all_names = in_names + out_names
    if part_name is not None:
        all_names = all_names + [part_name]

    def _body(*args):
        operands = list(args)
        if part_name is not None:
            operands.append(bass2jax.partition_id_tensor())
        outs = bass2jax._bass_exec_p.bind(
            *operands, out_avals=tuple(out_avals), in_names=tuple(all_names),
            out_names=tuple(out_names), lowering_input_output_aliases=(),
            sim_require_finite=False, sim_require_nnan=False, nc=nc)
        return tuple(outs)

    devices = jax.devices()[:n_cores]
    mesh = Mesh(np.asarray(devices), ("core",))
    in_specs = (PartitionSpec("core"),) * (n_params + len(out_names))
    out_specs = (PartitionSpec("core"),) * len(out_names)
    fn = jax.jit(shard_map(_body, mesh=mesh, in_specs=in_specs,
                           out_specs=out_specs, check_rep=False))
    concat_in = [np.concatenate([np.asarray(in_maps[c][nm])
                                 for c in range(n_cores)], axis=0)
                 for nm in in_names]
    concat_zeros = [np.zeros((n_cores * z.shape[0], *z.shape[1:]), z.dtype)
                    for z in zero_outs]
    dev_in = [jax.device_put(
        a, jax.sharding.NamedSharding(mesh, PartitionSpec("core")))
        for a in concat_in + concat_zeros]

    def run():
        outs = fn(*dev_in)
        outs = [np.asarray(o) for o in outs]
        return [
            {nm: outs[i].reshape(n_cores, *out_avals[i].shape)[c]
             for i, nm in enumerate(out_names)}
            for c in range(n_cores)]
    return run


def kernel(**inputs):
    """Full-input distributed GAT kernel; returns pooled [64] float32."""
    inputs = {k: np.asarray(v) for k, v in inputs.items()}
    N = inputs["x"].shape[0]
    sched, idx_arrs, d8_arrs, mt_arrs = preprocess(inputs["edge_index"], N)
    nc = build_program(sched)
    in_maps = make_inputs(sched, idx_arrs, d8_arrs, mt_arrs, inputs)
    run = make_runner(nc, in_maps)
    kernel.last_runner = run          # exposed for test.py timing
    kernel.last_inputs = inputs
    results = run()
    partial = sum(r["out64"][:, 0] for r in results)
    out = (partial + np.float32(inputs["lin_b"].reshape(-1)[0]))[:64]
    return out.astype(np.float32)

